# revision 93
# baseline (speedup 1.0000x reference)
"""Multi-head attention (QKV projections + causal/padded softmax attention +
output projection + residual + LayerNorm) as a Bass/Tile kernel on 8 Trainium2
cores — NO collectives.

Sharding: rows (sequence) are sharded across cores; every core computes ALL 16
heads for its own 512 rows end-to-end, so no cross-core communication is ever
needed.  Core c handles batch b = c//4 and the four 128-row tiles
t_j = 4*j + (c%4), j = 0..3 (interleaved so early/late causal tiles spread
evenly).  The price is that each 4-core batch group re-computes the batch's
K/V projections (up to kmax keys) redundantly; that costs ~34us of PE but
saves the ~100us collective chain (CC barrier + 2 AllToAlls) the head-sharded
variant pays.

SPMD trick for the causal mask: the program is identical on all cores, but the
position of the causal diagonal inside each row-tile's key loop is
core-dependent.  All key/row masking is therefore driven by per-core INPUT
constants applied along hardware-broadcast axes only:
  - bias_sb[key, (j, kb)]: per-key exp bias = 0 (valid) / -1e9 (key padded or
    chunk entirely above the diagonal); consumed as the scalar-activation
    per-partition bias of the fused exp, so masking is free.
  - alpha[(j, kb)]: per-core scalar that multiplies a static upper-triangle
    0/1 constant (TRIrep8, replicated per head) added onto the scores psum by
    ONE scalar_tensor_tensor per (row-tile, chunk) position where ANY core
    could have its diagonal: sc += alpha * TRI.  alpha = -1e9 exactly on this
    core's diagonal chunk, 0 elsewhere.
Scores live as sc[key, row] (key on partitions) so the pad mask is a
per-partition column; V is augmented with a ones column so row 64 of the ctx
psum accumulates the softmax denominators (no extra matmul).

Matmul cost on TRN2 = moving-column count (contraction/output width free), so
all operand layouts are chosen to minimize total moving columns:
Q 32.8k + K 8*8*kpad + V same + scores/ctx 16*sum(cap_j)*128 each + Wo 32.8k
cycles at 2.4 GHz (p-state held by a dense back-to-back PE stream).
"""

import math
from contextlib import ExitStack

import numpy as np
import ml_dtypes

import concourse.mybir as mybir
import concourse.tile as tile
from concourse import bacc
from concourse.bass_utils import run_bass_kernel_spmd

BF16 = mybir.dt.bfloat16
F32 = mybir.dt.float32
FP8 = mybir.dt.float8e4
W_SCALE = 8.0  # host scales W and x->e4m3; folded back via exp scale / ones

NEG_INF = -1e9
LN_EPS = 1e-6


class Cfg:
    def __init__(self, B=2, S=2048, D=1024, H=16, dh=64, kmax=None):
        self.B, self.S, self.D, self.H, self.dh = B, S, D, H, dh
        self.kmax = S if kmax is None else max(1, min(int(kmax), S))
        self.NC = 8                       # cores
        self.G = 4                        # cores per batch group
        self.RPC = S // self.G            # rows per core (512)
        self.NT = self.RPC // 128         # row-tiles per core (4)
        self.DC = D // 128                # contraction chunks (8)
        self.NP = H // 2                  # head pairs (8)
        self.KB = -(-self.kmax // 128)    # key chunks actually needed
        self.KPAD = self.KB * 128
        # slot j covers row tile 4*j+q (q = core quarter); the static chunk
        # cap must cover the deepest core (q=3)
        self.caps = [min(4 * j + 4, self.KB) for j in range(self.NT)]
        # (j, kb) positions where ANY core's causal diagonal can fall
        self.POS = [(j, kb) for j in range(self.NT)
                    for kb in range(self.caps[j])
                    if 4 * j <= kb <= 4 * j + 3]
        # runtime-detected LN specializations
        self.G1 = False
        self.B0 = False
        # per-projection fp8 (DoubleRow) selection.  All False: e4m3
        # projections measured 0.025 max-rel-err on HW (2x the simulator's
        # 0.013) against the 0.02 gate — not worth the risk for ~15us.
        self.FQ = False
        self.FK = False
        self.FV = False


def build_program(cfg: Cfg):
    nc = bacc.Bacc("TRN2", target_bir_lowering=False, debug=False,
                   num_devices=cfg.NC)

    D, dh = cfg.D, cfg.dh
    KB, KPAD, RPC, NT = cfg.KB, cfg.KPAD, cfg.RPC, cfg.NT

    # Selected projections are pre-quantized to e4m3 on the host (weights
    # scaled by W_SCALE) and consumed by DoubleRow matmuls: 256-deep
    # contraction per instruction at 0.5 cycles/row — half the HBM bytes,
    # half the PE instructions, and half the cycles of the bf16 path.
    DR = cfg.DC // 2

    def x_in(name, cols, f8):
        if f8:
            return nc.dram_tensor(name, [128, DR, 2, cols], FP8,
                                  kind="ExternalInput").ap()
        return nc.dram_tensor(name, [128, cfg.DC, cols], BF16,
                              kind="ExternalInput").ap()

    xq = x_in("xq", RPC, cfg.FQ)
    xk = x_in("xk", KPAD, cfg.FK)
    wqT = x_in("wqT", D, cfg.FQ)
    wkT = x_in("wkT", D, cfg.FK)
    wvT = x_in("wvT", D, cfg.FV)
    if cfg.FV:
        xv = nc.dram_tensor("xv", [KB, 128, DR, 2, 128], FP8,
                            kind="ExternalInput").ap()
    else:
        xv = nc.dram_tensor("xv", [KB, 128, cfg.DC, 128], BF16,
                            kind="ExternalInput").ap()
    woT = nc.dram_tensor("woT", [128, cfg.DC, D], BF16,
                         kind="ExternalInput").ap()
    # per-core post-exp mask: pmask[key, kb, h2*512 + j*128 + f] in {0,1}
    # covers pad (key >= sen_len), causal (row < key), and above-diagonal
    # chunks, identical for every head; the 512-row block is stored twice
    # (h2 = 0/1) so one [128, 1024] multiply masks a whole pair tile.
    pmask = nc.dram_tensor("pmask", [128, KB, 1024], BF16,
                           kind="ExternalInput").ap()
    resid = nc.dram_tensor("resid", [128, NT, D], BF16,
                           kind="ExternalInput").ap()
    gamma = nc.dram_tensor("gamma", [1, D], BF16, kind="ExternalInput").ap()
    beta = nc.dram_tensor("beta", [1, D], BF16, kind="ExternalInput").ap()
    out_shard = nc.dram_tensor("out_shard", [RPC, D], BF16,
                               kind="ExternalOutput").ap()

    with tile.TileContext(nc) as tc, ExitStack() as ctx:
        consts = ctx.enter_context(tc.tile_pool(name="consts", bufs=1))
        xin = ctx.enter_context(tc.tile_pool(name="xin", bufs=1))
        proj = ctx.enter_context(tc.tile_pool(name="proj", bufs=1))
        att = ctx.enter_context(tc.tile_pool(name="att", bufs=2))
        small = ctx.enter_context(tc.tile_pool(name="small", bufs=2))
        lnp = ctx.enter_context(tc.tile_pool(name="lnp", bufs=2))
        psum = ctx.enter_context(
            tc.tile_pool(name="psum", bufs=1, space="PSUM"))

        # ---- constants -----------------------------------------------------
        # weights ride the scalar engine's DMA queue (parallel to the sync
        # queue that streams activations); per-dc chunks so the first
        # accumulation chain can start after 1/8 of the weight transfer.
        def w_tile(name, f8):
            if f8:
                return consts.tile([128, DR, 2, D], FP8, name=name)
            return consts.tile([128, cfg.DC, D], BF16, name=name)

        wk_sb = w_tile("wk_sb", cfg.FK)
        wq_sb = w_tile("wq_sb", cfg.FQ)
        wv_sb = w_tile("wv_sb", cfg.FV)
        for w_sb, w_dram in ((wk_sb, wkT), (wq_sb, wqT), (wv_sb, wvT)):
            for i in range(w_sb.shape[1]):
                nc.scalar.dma_start(out=w_sb[:, i], in_=w_dram[:, i])



        # P3 constants (loaded later on the sync queue, behind the inputs)
        gamma_bc = beta_bc = None
        if not (cfg.G1 and cfg.B0):
            g_row = consts.tile([1, D], BF16)
            b_row = consts.tile([1, D], BF16)
            gamma_bc = consts.tile([128, D], BF16)
            beta_bc = consts.tile([128, D], BF16)
        eps_sb = consts.tile([128, 1], F32)
        nc.vector.memset(eps_sb, LN_EPS)
        res_sb = consts.tile([128, NT, D], BF16)

        # ---- P1: projections ----------------------------------------------
        khT_sb = proj.tile([128, cfg.NP, KPAD], BF16)
        qhT_sb = proj.tile([128, cfg.NP, RPC], BF16)
        vh_sb = proj.tile([128, KB, cfg.H, dh + 1], BF16)
        # the ones column carries W_SCALE so ctx/den cancels the vh scale
        nc.gpsimd.memset(vh_sb[:, :, :, dh:dh + 1],
                         W_SCALE if cfg.FV else 1.0)

        # per-chunk DMAs (contiguous per partition): the first K matmul
        # only waits on chunk 0, not the whole transfer
        def x_tile(name, cols, f8):
            if f8:
                t = xin.tile([128, DR, 2, cols], FP8, name=name)
            else:
                t = xin.tile([128, cfg.DC, cols], BF16, name=name)
            return t

        xk_sb = x_tile("xk_sb", KPAD, cfg.FK)
        for i in range(xk_sb.shape[1]):
            nc.sync.dma_start(out=xk_sb[:, i], in_=xk[:, i])
        xq_sb = x_tile("xq_sb", RPC, cfg.FQ)
        for i in range(xq_sb.shape[1]):
            nc.sync.dma_start(out=xq_sb[:, i], in_=xq[:, i])
        xv_sb = []
        for kb in range(KB):
            if cfg.FV:
                t = xin.tile([128, DR, 2, 128], FP8, tag="xv", bufs=KB,
                             name=f"xv{kb}")
            else:
                t = xin.tile([128, cfg.DC, 128], BF16, tag="xv", bufs=KB,
                             name=f"xv{kb}")
            nc.sync.dma_start(out=t, in_=xv[kb])
            xv_sb.append(t)

        copy_flip = [0]

        def pcopy(out_ap, in_ap):
            # alternate psum->sbuf copies between DVE and the scalar engine
            eng = nc.vector if copy_flip[0] % 2 == 0 else nc.scalar
            copy_flip[0] += 1
            if eng is nc.vector:
                eng.tensor_copy(out=out_ap, in_=in_ap)
            else:
                eng.copy(out=out_ap, in_=in_ap)

        DRM = mybir.MatmulPerfMode.DoubleRow

        def xw_proj(x_sb, w_sb, out_sb, width, f8):
            # fp8: lhsT = w [128, 2, 128] (two stacked k-tiles), rhs = x
            # [128, 2, cols]: 256-deep contraction per DoubleRow matmul
            nch = DR if f8 else cfg.DC
            for p in range(cfg.NP):
                for c0 in range(0, width, 512):
                    cw = min(512, width - c0)
                    ps = psum.tile([128, cw], F32, tag="ctx", bufs=4,
                                   name="ps_proj")
                    for i in range(nch):
                        if f8:
                            nc.tensor.matmul(
                                ps, w_sb[:, i, :, p * 128:(p + 1) * 128],
                                x_sb[:, i, :, c0:c0 + cw],
                                start=i == 0, stop=i == nch - 1,
                                perf_mode=DRM)
                        else:
                            nc.tensor.matmul(
                                ps, w_sb[:, i, p * 128:(p + 1) * 128],
                                x_sb[:, i, c0:c0 + cw],
                                start=i == 0, stop=i == nch - 1)
                    pcopy(out_sb[:, p, c0:c0 + cw], ps)

        # K first (scores need it first), then Q.  The V projections are
        # deferred: one V unit is emitted after each of the first two
        # pairs' score/exp iterations, so the scalar engine's softmax
        # backlog overlaps the PE's V work.  V uses the "ctx" psum ring,
        # which holds no live ctx tiles yet (they allocate lazily in
        # emit_ctx, after V), so ring reuse stays acyclic.
        xw_proj(xk_sb, wk_sb, khT_sb, KPAD, cfg.FK)
        xw_proj(xq_sb, wq_sb, qhT_sb, RPC, cfg.FQ)

        nchv = DR if cfg.FV else cfg.DC

        def v_unit(kb, half):
            def run():
                ps = psum.tile([128, 512], F32, tag="ctx", bufs=4,
                               name="ps_v")
                for i in range(nchv):
                    if cfg.FV:
                        nc.tensor.matmul(
                            ps, xv_sb[kb][:, i],
                            wv_sb[:, i, :, half * 512:half * 512 + 512],
                            start=i == 0, stop=i == nchv - 1,
                            perf_mode=DRM)
                    else:
                        nc.tensor.matmul(
                            ps, xv_sb[kb][:, i],
                            wv_sb[:, i, half * 512:half * 512 + 512],
                            start=i == 0, stop=i == nchv - 1)
                pcopy(
                    vh_sb[:, kb, 8 * half:8 * half + 8, 0:dh],
                    ps.rearrange("p (h e) -> p h e", e=dh))
            return run

        v_units = [v_unit(kb, half) for kb in range(KB) for half in range(2)]

        # late transfers, in need order: pmask (first diag ~30us), then wo
        # (needed ~130us), then the LN constants — all behind the
        # critical-path weight/input streams
        pm_sb = consts.tile([128, KB, 1024], BF16)
        nc.sync.dma_start(out=pm_sb, in_=pmask)
        wo_sb = consts.tile([128, cfg.DC, D], BF16, name="wo_sb")
        for dc in range(cfg.DC):
            nc.scalar.dma_start(out=wo_sb[:, dc, :], in_=woT[:, dc, :])
        nc.sync.dma_start(out=res_sb, in_=resid)
        if gamma_bc is not None:
            nc.sync.dma_start(out=g_row, in_=gamma)
            nc.sync.dma_start(out=b_row, in_=beta)
            nc.gpsimd.partition_broadcast(gamma_bc, g_row)
            nc.gpsimd.partition_broadcast(beta_bc, b_row)

        # ---- P2: attention (pair-major, all 512 rows per tile) -------------
        ctxT_sb = proj.tile([128, cfg.NP, RPC], BF16)

        def divide(p, ctx_ps):
            # ctx rows 0..63 / den row 64; stage ctxT = ctx/den for Wo
            for h2 in range(2):
                den = small.tile([1, 512], F32, tag=f"den{h2}", bufs=2,
                                 name=f"den{h2}")
                nc.vector.tensor_copy(out=den, in_=ctx_ps[h2][dh:dh + 1, :])
                rec = small.tile([1, 512], F32, tag=f"rec{h2}", bufs=2,
                                 name=f"rec{h2}")
                nc.vector.reciprocal_approx_fast(rec, den)
                rbc = small.tile([64, 512], F32, tag=f"rbc{h2}", bufs=2,
                                 name=f"rbc{h2}")
                nc.gpsimd.partition_broadcast(rbc, rec)
                nc.vector.tensor_mul(
                    ctxT_sb[64 * h2:64 * h2 + 64, p, :],
                    ctx_ps[h2][0:dh, :], rbc)

        ctx_tiles = {}

        def emit_ctx(entry):
            kb, p, probs = entry
            if p not in ctx_tiles:
                ctx_tiles[p] = [psum.tile([dh + 1, 512], F32, tag="ctx",
                                          bufs=4, name=f"ctx{h2}")
                                for h2 in range(2)]
            ctx_ps = ctx_tiles[p]
            for h2 in range(2):
                nc.tensor.matmul(
                    ctx_ps[h2], vh_sb[:, kb, 2 * p + h2, :],
                    probs[:, h2 * 512:(h2 + 1) * 512],
                    start=kb == 0, stop=kb == KB - 1)
            if kb == KB - 1:
                divide(p, ctx_ps)
                del ctx_tiles[p]

        pend = []

        def att_iter(p, kb):
            sc = psum.tile([128, 1024], F32, tag="sc", bufs=2, name="sc")
            for h2 in range(2):
                lo = 64 * h2
                nc.tensor.matmul(
                    sc[:, h2 * 512:(h2 + 1) * 512],
                    khT_sb[lo:lo + 64, p, kb * 128:(kb + 1) * 128],
                    qhT_sb[lo:lo + 64, p, :],
                    start=True, stop=True)
            probs = att.tile([128, 1024], BF16, tag="pr", bufs=12,
                             name="probs")
            nc.scalar.activation(
                out=probs, in_=sc,
                func=mybir.ActivationFunctionType.Exp,
                scale=1.0 / math.sqrt(dh)
                / (W_SCALE if cfg.FQ else 1.0)
                / (W_SCALE if cfg.FK else 1.0))
            nc.gpsimd.tensor_mul(probs, probs, pm_sb[:, kb, :])
            pend.append((kb, p, probs))

        for p in range(cfg.NP):
            for kb in range(KB):
                att_iter(p, kb)
                if v_units:
                    v_units.pop(0)()
                else:
                    for _ in range(2):
                        if len(pend) > 2:
                            emit_ctx(pend.pop(0))
        for entry in pend:
            emit_ctx(entry)

        # ---- P3: Wo + residual + LayerNorm ---------------------------------
        for rt in range(NT):
            pso = [psum.tile([128, 512], F32, tag="ctx", bufs=4,
                             name=f"pso{ns}") for ns in range(2)]
            for p in range(cfg.NP):
                for ns in range(2):
                    nc.tensor.matmul(
                        pso[ns], ctxT_sb[:, p, rt * 128:(rt + 1) * 128],
                        wo_sb[:, p, ns * 512:ns * 512 + 512],
                        start=p == 0, stop=p == cfg.NP - 1)
            x = lnp.tile([128, D], F32, tag="x")
            for ns in range(2):
                nc.vector.tensor_add(x[:, ns * 512:ns * 512 + 512], pso[ns],
                                     res_sb[:, rt, ns * 512:ns * 512 + 512])
            fmax = math.gcd(nc.vector.BN_STATS_FMAX, D)
            nsub = D // fmax
            stats = lnp.tile([128, nsub, nc.vector.BN_STATS_DIM], F32,
                             tag="stats")
            for sg in range(nsub):
                nc.vector.bn_stats(
                    out=stats[:, sg, :],
                    in_=x.rearrange("p (a b) -> p a b", a=nsub)[:, sg, :])
            mv = lnp.tile([128, nc.vector.BN_AGGR_DIM], F32, tag="mv")
            nc.vector.bn_aggr(out=mv, in_=stats)
            sd = lnp.tile([128, 1], F32, tag="sd")
            nc.scalar.activation(out=sd, in_=mv[:, 1:2],
                                 func=mybir.ActivationFunctionType.Sqrt,
                                 bias=eps_sb, scale=1.0)
            rstd = lnp.tile([128, 1], F32, tag="rstd")
            nc.vector.reciprocal_approx_fast(rstd, sd)
            out_sb = lnp.tile([128, D], BF16, tag="out_sb")
            if cfg.G1 and cfg.B0:
                nc.vector.tensor_scalar(
                    out=out_sb, in0=x, scalar1=mv[:, 0:1], scalar2=rstd,
                    op0=mybir.AluOpType.subtract, op1=mybir.AluOpType.mult)
            else:
                y = lnp.tile([128, D], BF16, tag="y")
                nc.vector.tensor_scalar(
                    out=y, in0=x, scalar1=mv[:, 0:1], scalar2=rstd,
                    op0=mybir.AluOpType.subtract, op1=mybir.AluOpType.mult)
                if cfg.B0:
                    nc.vector.tensor_mul(out_sb, y, gamma_bc)
                elif cfg.G1:
                    nc.vector.tensor_add(out_sb, y, beta_bc)
                else:
                    yg = lnp.tile([128, D], BF16, tag="yg")
                    nc.vector.tensor_mul(yg, y, gamma_bc)
                    nc.vector.tensor_add(out_sb, yg, beta_bc)
            nc.sync.dma_start(out=out_shard[rt * 128:(rt + 1) * 128, :],
                              in_=out_sb)

    nc.compile()
    return nc


def _tile_x8(xT):
    """[D, C] f32 -> [128, D/256, 2, C] e4m3 (DoubleRow k-tile layout)."""
    d, c = xT.shape
    t = xT.reshape(d // 256, 2, 128, c).transpose(2, 0, 1, 3)
    return np.ascontiguousarray(t.astype(ml_dtypes.float8_e4m3fn))


def _tile_w(wT):
    d, o = wT.shape
    return np.ascontiguousarray(
        wT.reshape(d // 128, 128, o).transpose(1, 0, 2))


def make_in_maps(cfg: Cfg, q, k, v, Wq, Wk, Wv, Wo, gamma, beta, sen_len):
    bf = ml_dtypes.bfloat16
    q = np.asarray(q, np.float32)
    k = np.asarray(k, np.float32)
    v = np.asarray(v, np.float32)
    # fp8 weights are scaled by W_SCALE before the e4m3 cast (avoids
    # subnormals); the scale is cancelled by the exp scale (q,k) / the
    # scaled ones column (v)
    def tile_xw(xT, f8):
        return _tile_x8(xT * W_SCALE) if f8 else _tile_w(xT.astype(bf))

    wq_t = tile_xw(np.asarray(Wq, np.float32).T, cfg.FQ)
    wk_t = tile_xw(np.asarray(Wk, np.float32).T, cfg.FK)
    wv_t = tile_xw(np.asarray(Wv, np.float32).T, cfg.FV)
    wo_t = _tile_w(np.asarray(Wo, np.float32).T.astype(bf))
    g_row = np.asarray(gamma, np.float32).reshape(1, cfg.D).astype(bf)
    b_row = np.asarray(beta, np.float32).reshape(1, cfg.D).astype(bf)

    KB, KPAD, NT = cfg.KB, cfg.KPAD, cfg.NT
    per_batch = {}
    for b in range(cfg.B):
        kT_t = _tile_x8(k[b, :KPAD, :].T) if cfg.FK \
            else _tile_w(k[b, :KPAD, :].T.astype(bf))
        vT = v[b, :KPAD, :].T                     # [D, KPAD]
        if cfg.FV:
            xv = np.ascontiguousarray(
                vT.reshape(4, 2, 128, KB, 128).transpose(3, 2, 0, 1, 4)
                .astype(ml_dtypes.float8_e4m3fn))  # [KB, 128, DR, 2, 128]
        else:
            xv = np.ascontiguousarray(
                vT.astype(bf).reshape(8, 128, KB, 128)
                .transpose(2, 1, 0, 3))            # [KB, 128, DC, 128]
        per_batch[b] = (kT_t, xv)

    key_pos = np.arange(KPAD)
    in_maps = []
    for c in range(cfg.NC):
        b, qq = c // cfg.G, c % cfg.G
        tiles = [4 * j + qq for j in range(NT)]
        rows = np.concatenate(
            [np.arange(t * 128, (t + 1) * 128) for t in tiles])
        sl = int(np.asarray(sen_len)[b])

        # post-exp multiplicative mask pmask[key_p, kb, h2*512 + j*128 + f]:
        # 1 iff row (= tiles[j]*128 + f) >= key (= kb*128 + key_p) and
        # key < sen_len; identical for both h2 halves.
        rows_g = np.concatenate(
            [tiles[j] * 128 + np.arange(128) for j in range(NT)])  # [512]
        keys_g = (np.arange(KB)[:, None] * 128
                  + np.arange(128)[None, :])                       # [KB,128]
        valid = ((rows_g[None, None, :] >= keys_g[:, :, None])
                 & (keys_g[:, :, None] < sl))                      # [KB,128,512]
        pm = np.broadcast_to(
            valid.transpose(1, 0, 2)[:, :, None, :],
            (128, KB, 2, 512)).reshape(128, KB, 1024)
        pm = np.ascontiguousarray(pm.astype(ml_dtypes.bfloat16))

        xq_h = _tile_x8(q[b][rows, :].T) if cfg.FQ \
            else _tile_w(q[b][rows, :].T.astype(bf))
        res = np.ascontiguousarray(
            q[b][rows, :].reshape(NT, 128, cfg.D)
            .transpose(1, 0, 2).astype(bf))
        kT_t, xv_t = per_batch[b]
        in_maps.append({
            "xq": xq_h, "xk": kT_t, "xv": xv_t,
            "wqT": wq_t, "wkT": wk_t, "wvT": wv_t, "woT": wo_t,
            "pmask": pm, "resid": res,
            "gamma": g_row, "beta": b_row,
        })
    return in_maps


def assemble_output(cfg: Cfg, results):
    out = np.empty((cfg.B, cfg.S, cfg.D), np.float32)
    for c in range(cfg.NC):
        b, qq = c // cfg.G, c % cfg.G
        shard = results[c]["out_shard"].astype(np.float32)
        for j in range(cfg.NT):
            t = 4 * j + qq
            out[b, t * 128:(t + 1) * 128, :] = shard[j * 128:(j + 1) * 128]
    return out


_PROGRAM_CACHE = {}


def _get_program(cfg: Cfg):
    key = (cfg.B, cfg.S, cfg.D, cfg.H, cfg.dh, cfg.KB, cfg.G1, cfg.B0,
           cfg.FQ, cfg.FK, cfg.FV)
    if key not in _PROGRAM_CACHE:
        _PROGRAM_CACHE[key] = build_program(cfg)
    return _PROGRAM_CACHE[key]


def run(cfg: Cfg, inputs: dict, trace: bool = False):
    cfg.G1 = bool(np.all(np.asarray(inputs["gamma"]) == 1.0))
    cfg.B0 = bool(np.all(np.asarray(inputs["beta"]) == 0.0))
    nc = _get_program(cfg)
    in_maps = make_in_maps(cfg, **inputs)
    res = run_bass_kernel_spmd(nc, in_maps, core_ids=list(range(cfg.NC)),
                               trace=trace)
    return assemble_output(cfg, res.results), res


def kernel(**inputs) -> np.ndarray:
    kmax = int(np.max(inputs["sen_len"]))
    cfg = Cfg(B=2, S=2048, D=1024, H=16, dh=64, kmax=kmax)
    out, _ = run(cfg, inputs)
    return out


# revision 95
# speedup vs baseline: 1.1050x; 1.1050x over previous
"""Multi-head attention (QKV projections + causal/padded softmax attention +
output projection + residual + LayerNorm) as a Bass/Tile kernel on 8 Trainium2
cores — NO collectives.

Sharding: rows (sequence) are sharded across cores; every core computes ALL 16
heads for its own 512 rows end-to-end, so no cross-core communication is ever
needed.  Core c handles batch b = c//4 and the four 128-row tiles
t_j = 4*j + (c%4), j = 0..3.  The price is that each 4-core batch group
re-computes the batch's K/V projections (up to kmax keys) redundantly; that
costs ~25us of PE but saves the ~100us collective chain (CC-stream barrier +
2 AllToAlls) the head-sharded variant pays in this environment.

SPMD trick for the causal mask: the program is identical on all cores, but
the causal-diagonal position is core-dependent (rows differ per core).  ALL
masking (key padding, whole chunks above the diagonal, and the in-diagonal
triangle) is therefore one multiplicative 0/1 bf16 mask per key-chunk,
precomputed PER CORE on the host and applied to the probs AFTER the exp on
the (otherwise idle) GpSimd engine — nothing mask-related ever touches the
scores->exp critical chain, and exp of unmasked scores is safe (|s|/8 is a
few units at most).  V is augmented with a ones column so row 64 of the ctx
psum accumulates the softmax denominators for free.

Attention is pair-major: per (head-pair, key-chunk) one 512-row score matmul
per head half into separate psum banks (a hard HW constraint: matmuls whose
stationary sits at different partition bases — PE row-tile 0 vs 64 — must
not write the same PSUM bank, or the NEFF dies at runtime), one [128,1024]
exp, and two 512-row ctx matmuls accumulated over chunks.  512-wide moving
operands matter: matmul wall time is ~(moving x 0.42ns + ~145ns overhead +
LDWEIGHTS), so few big matmuls beat many small ones.

All engines are load-balanced: PE ~120us of matmul, scalar exp ~48us + half
the psum->sbuf copies, DVE the other copies + denominators + LayerNorm,
GpSimd the masks/broadcasts.  Weights stream per-dc-chunk on the scalar DMA
queue in first-use order (wk, wq, wv, then wo late); activations on the sync
queue; masks/residual/LN constants behind them.  fp8(e4m3) DoubleRow
projections were tried and measured 0.025 max-rel-err on hardware (2x the
simulator estimate) against the 0.02 gate, so everything stays bf16.
"""

import math
from contextlib import ExitStack

import numpy as np
import ml_dtypes

import concourse.mybir as mybir
import concourse.tile as tile
from concourse import bacc
from concourse.bass_utils import run_bass_kernel_spmd

BF16 = mybir.dt.bfloat16
F32 = mybir.dt.float32
FP8 = mybir.dt.float8e4
W_SCALE = 8.0  # host scales W and x->e4m3; folded back via exp scale / ones

NEG_INF = -1e9
LN_EPS = 1e-6


class Cfg:
    def __init__(self, B=2, S=2048, D=1024, H=16, dh=64, kmax=None):
        self.B, self.S, self.D, self.H, self.dh = B, S, D, H, dh
        self.kmax = S if kmax is None else max(1, min(int(kmax), S))
        self.NC = 8                       # cores
        self.G = 4                        # cores per batch group
        self.RPC = S // self.G            # rows per core (512)
        self.NT = self.RPC // 128         # row-tiles per core (4)
        self.DC = D // 128                # contraction chunks (8)
        self.NP = H // 2                  # head pairs (8)
        self.KB = -(-self.kmax // 128)    # key chunks actually needed
        self.KPAD = self.KB * 128
        # slot j covers row tile 4*j+q (q = core quarter); the static chunk
        # cap must cover the deepest core (q=3)
        self.caps = [min(4 * j + 4, self.KB) for j in range(self.NT)]
        # (j, kb) positions where ANY core's causal diagonal can fall
        self.POS = [(j, kb) for j in range(self.NT)
                    for kb in range(self.caps[j])
                    if 4 * j <= kb <= 4 * j + 3]
        # runtime-detected LN specializations
        self.G1 = False
        self.B0 = False
        # per-projection fp8 (DoubleRow) selection.  All False: e4m3
        # projections measured 0.025 max-rel-err on HW (2x the simulator's
        # 0.013) against the 0.02 gate — not worth the risk for ~15us.
        self.FQ = False
        self.FK = False
        self.FV = False


def build_program(cfg: Cfg):
    nc = bacc.Bacc("TRN2", target_bir_lowering=False, debug=False,
                   num_devices=cfg.NC)

    D, dh = cfg.D, cfg.dh
    KB, KPAD, RPC, NT = cfg.KB, cfg.KPAD, cfg.RPC, cfg.NT

    # Selected projections are pre-quantized to e4m3 on the host (weights
    # scaled by W_SCALE) and consumed by DoubleRow matmuls: 256-deep
    # contraction per instruction at 0.5 cycles/row — half the HBM bytes,
    # half the PE instructions, and half the cycles of the bf16 path.
    DR = cfg.DC // 2

    def x_in(name, cols, f8):
        if f8:
            return nc.dram_tensor(name, [128, DR, 2, cols], FP8,
                                  kind="ExternalInput").ap()
        return nc.dram_tensor(name, [128, cfg.DC, cols], BF16,
                              kind="ExternalInput").ap()

    xq = x_in("xq", RPC, cfg.FQ)
    xk = x_in("xk", KPAD, cfg.FK)
    wqT = x_in("wqT", D, cfg.FQ)
    wkT = x_in("wkT", D, cfg.FK)
    wvT = x_in("wvT", D, cfg.FV)
    if cfg.FV:
        xv = nc.dram_tensor("xv", [KB, 128, DR, 2, 128], FP8,
                            kind="ExternalInput").ap()
    else:
        xv = nc.dram_tensor("xv", [KB, 128, cfg.DC, 128], BF16,
                            kind="ExternalInput").ap()
    woT = nc.dram_tensor("woT", [128, cfg.DC, D], BF16,
                         kind="ExternalInput").ap()
    # per-core post-exp mask: pmask[key, kb, h2*512 + j*128 + f] in {0,1}
    # covers pad (key >= sen_len), causal (row < key), and above-diagonal
    # chunks, identical for every head; the 512-row block is stored twice
    # (h2 = 0/1) so one [128, 1024] multiply masks a whole pair tile.
    pmask = nc.dram_tensor("pmask", [128, KB, 1024], BF16,
                           kind="ExternalInput").ap()
    resid = nc.dram_tensor("resid", [128, NT, D], BF16,
                           kind="ExternalInput").ap()
    gamma = nc.dram_tensor("gamma", [1, D], BF16, kind="ExternalInput").ap()
    beta = nc.dram_tensor("beta", [1, D], BF16, kind="ExternalInput").ap()
    out_shard = nc.dram_tensor("out_shard", [RPC, D], BF16,
                               kind="ExternalOutput").ap()

    with tile.TileContext(nc) as tc, ExitStack() as ctx:
        consts = ctx.enter_context(tc.tile_pool(name="consts", bufs=1))
        xin = ctx.enter_context(tc.tile_pool(name="xin", bufs=1))
        proj = ctx.enter_context(tc.tile_pool(name="proj", bufs=1))
        att = ctx.enter_context(tc.tile_pool(name="att", bufs=2))
        small = ctx.enter_context(tc.tile_pool(name="small", bufs=2))
        lnp = ctx.enter_context(tc.tile_pool(name="lnp", bufs=2))
        psum = ctx.enter_context(
            tc.tile_pool(name="psum", bufs=1, space="PSUM"))

        # ---- constants -----------------------------------------------------
        # weights ride the scalar engine's DMA queue (parallel to the sync
        # queue that streams activations); per-dc chunks so the first
        # accumulation chain can start after 1/8 of the weight transfer.
        def w_tile(name, f8):
            if f8:
                return consts.tile([128, DR, 2, D], FP8, name=name)
            return consts.tile([128, cfg.DC, D], BF16, name=name)

        wk_sb = w_tile("wk_sb", cfg.FK)
        wq_sb = w_tile("wq_sb", cfg.FQ)
        wv_sb = w_tile("wv_sb", cfg.FV)
        for w_sb, w_dram in ((wk_sb, wkT), (wq_sb, wqT), (wv_sb, wvT)):
            for i in range(w_sb.shape[1]):
                nc.scalar.dma_start(out=w_sb[:, i], in_=w_dram[:, i])



        # P3 constants (loaded later on the sync queue, behind the inputs)
        gamma_bc = beta_bc = None
        if not (cfg.G1 and cfg.B0):
            g_row = consts.tile([1, D], BF16)
            b_row = consts.tile([1, D], BF16)
            gamma_bc = consts.tile([128, D], BF16)
            beta_bc = consts.tile([128, D], BF16)
        eps_sb = consts.tile([128, 1], F32)
        nc.vector.memset(eps_sb, LN_EPS)
        res_sb = consts.tile([128, NT, D], BF16)

        # ---- P1: projections ----------------------------------------------
        khT_sb = proj.tile([128, cfg.NP, KPAD], BF16)
        qhT_sb = proj.tile([128, cfg.NP, RPC], BF16)
        vh_sb = proj.tile([128, KB, cfg.H, dh + 1], BF16)
        # the ones column carries W_SCALE so ctx/den cancels the vh scale
        nc.gpsimd.memset(vh_sb[:, :, :, dh:dh + 1],
                         W_SCALE if cfg.FV else 1.0)

        # per-chunk DMAs (contiguous per partition): the first K matmul
        # only waits on chunk 0, not the whole transfer
        def x_tile(name, cols, f8):
            if f8:
                t = xin.tile([128, DR, 2, cols], FP8, name=name)
            else:
                t = xin.tile([128, cfg.DC, cols], BF16, name=name)
            return t

        xk_sb = x_tile("xk_sb", KPAD, cfg.FK)
        for i in range(xk_sb.shape[1]):
            nc.sync.dma_start(out=xk_sb[:, i], in_=xk[:, i])
        xq_sb = x_tile("xq_sb", RPC, cfg.FQ)
        for i in range(xq_sb.shape[1]):
            nc.sync.dma_start(out=xq_sb[:, i], in_=xq[:, i])
        xv_sb = []
        for kb in range(KB):
            if cfg.FV:
                t = xin.tile([128, DR, 2, 128], FP8, tag="xv", bufs=KB,
                             name=f"xv{kb}")
            else:
                t = xin.tile([128, cfg.DC, 128], BF16, tag="xv", bufs=KB,
                             name=f"xv{kb}")
            nc.sync.dma_start(out=t, in_=xv[kb])
            xv_sb.append(t)

        copy_flip = [0]

        def pcopy(out_ap, in_ap):
            # alternate psum->sbuf copies between DVE and the scalar engine
            eng = nc.vector if copy_flip[0] % 2 == 0 else nc.scalar
            copy_flip[0] += 1
            if eng is nc.vector:
                eng.tensor_copy(out=out_ap, in_=in_ap)
            else:
                eng.copy(out=out_ap, in_=in_ap)

        DRM = mybir.MatmulPerfMode.DoubleRow

        def xw_proj(x_sb, w_sb, out_sb, width, f8):
            # fp8: lhsT = w [128, 2, 128] (two stacked k-tiles), rhs = x
            # [128, 2, cols]: 256-deep contraction per DoubleRow matmul
            nch = DR if f8 else cfg.DC
            for p in range(cfg.NP):
                for c0 in range(0, width, 512):
                    cw = min(512, width - c0)
                    ps = psum.tile([128, cw], F32, tag="ctx", bufs=4,
                                   name="ps_proj")
                    for i in range(nch):
                        if f8:
                            nc.tensor.matmul(
                                ps, w_sb[:, i, :, p * 128:(p + 1) * 128],
                                x_sb[:, i, :, c0:c0 + cw],
                                start=i == 0, stop=i == nch - 1,
                                perf_mode=DRM)
                        else:
                            nc.tensor.matmul(
                                ps, w_sb[:, i, p * 128:(p + 1) * 128],
                                x_sb[:, i, c0:c0 + cw],
                                start=i == 0, stop=i == nch - 1)
                    pcopy(out_sb[:, p, c0:c0 + cw], ps)

        # K first (scores need it first), then Q.  The V projections are
        # deferred: one V unit is emitted after each of the first two
        # pairs' score/exp iterations, so the scalar engine's softmax
        # backlog overlaps the PE's V work.  V uses the "ctx" psum ring,
        # which holds no live ctx tiles yet (they allocate lazily in
        # emit_ctx, after V), so ring reuse stays acyclic.
        xw_proj(xk_sb, wk_sb, khT_sb, KPAD, cfg.FK)
        xw_proj(xq_sb, wq_sb, qhT_sb, RPC, cfg.FQ)

        nchv = DR if cfg.FV else cfg.DC

        def v_unit(kb, half):
            def run():
                ps = psum.tile([128, 512], F32, tag="ctx", bufs=4,
                               name="ps_v")
                for i in range(nchv):
                    if cfg.FV:
                        nc.tensor.matmul(
                            ps, xv_sb[kb][:, i],
                            wv_sb[:, i, :, half * 512:half * 512 + 512],
                            start=i == 0, stop=i == nchv - 1,
                            perf_mode=DRM)
                    else:
                        nc.tensor.matmul(
                            ps, xv_sb[kb][:, i],
                            wv_sb[:, i, half * 512:half * 512 + 512],
                            start=i == 0, stop=i == nchv - 1)
                pcopy(
                    vh_sb[:, kb, 8 * half:8 * half + 8, 0:dh],
                    ps.rearrange("p (h e) -> p h e", e=dh))
            return run

        v_units = [v_unit(kb, half) for kb in range(KB) for half in range(2)]

        # late transfers, in need order: pmask (first diag ~30us), then wo
        # (needed ~130us), then the LN constants — all behind the
        # critical-path weight/input streams
        pm_sb = consts.tile([128, KB, 1024], BF16)
        nc.sync.dma_start(out=pm_sb, in_=pmask)
        wo_sb = consts.tile([128, cfg.DC, D], BF16, name="wo_sb")
        for dc in range(cfg.DC):
            nc.scalar.dma_start(out=wo_sb[:, dc, :], in_=woT[:, dc, :])
        nc.sync.dma_start(out=res_sb, in_=resid)
        if gamma_bc is not None:
            nc.sync.dma_start(out=g_row, in_=gamma)
            nc.sync.dma_start(out=b_row, in_=beta)
            nc.gpsimd.partition_broadcast(gamma_bc, g_row)
            nc.gpsimd.partition_broadcast(beta_bc, b_row)

        # ---- P2: attention (pair-major, all 512 rows per tile) -------------
        ctxT_sb = proj.tile([128, cfg.NP, RPC], BF16)

        def divide(p, ctx_ps):
            # ctx rows 0..63 / den row 64; stage ctxT = ctx/den for Wo
            for h2 in range(2):
                den = small.tile([1, 512], F32, tag=f"den{h2}", bufs=2,
                                 name=f"den{h2}")
                nc.vector.tensor_copy(out=den, in_=ctx_ps[h2][dh:dh + 1, :])
                rec = small.tile([1, 512], F32, tag=f"rec{h2}", bufs=2,
                                 name=f"rec{h2}")
                nc.vector.reciprocal_approx_fast(rec, den)
                rbc = small.tile([64, 512], F32, tag=f"rbc{h2}", bufs=2,
                                 name=f"rbc{h2}")
                nc.gpsimd.partition_broadcast(rbc, rec)
                nc.vector.tensor_mul(
                    ctxT_sb[64 * h2:64 * h2 + 64, p, :],
                    ctx_ps[h2][0:dh, :], rbc)

        ctx_tiles = {}

        def emit_ctx(entry):
            kb, p, probs = entry
            if p not in ctx_tiles:
                ctx_tiles[p] = [psum.tile([dh + 1, 512], F32, tag="ctx",
                                          bufs=4, name=f"ctx{h2}")
                                for h2 in range(2)]
            ctx_ps = ctx_tiles[p]
            for h2 in range(2):
                nc.tensor.matmul(
                    ctx_ps[h2], vh_sb[:, kb, 2 * p + h2, :],
                    probs[:, h2 * 512:(h2 + 1) * 512],
                    start=kb == 0, stop=kb == KB - 1)
            if kb == KB - 1:
                divide(p, ctx_ps)
                del ctx_tiles[p]

        pend = []

        def att_iter(p, kb):
            sc = psum.tile([128, 1024], F32, tag="sc", bufs=2, name="sc")
            for h2 in range(2):
                lo = 64 * h2
                nc.tensor.matmul(
                    sc[:, h2 * 512:(h2 + 1) * 512],
                    khT_sb[lo:lo + 64, p, kb * 128:(kb + 1) * 128],
                    qhT_sb[lo:lo + 64, p, :],
                    start=True, stop=True)
            probs = att.tile([128, 1024], BF16, tag="pr", bufs=12,
                             name="probs")
            nc.scalar.activation(
                out=probs, in_=sc,
                func=mybir.ActivationFunctionType.Exp,
                scale=1.0 / math.sqrt(dh)
                / (W_SCALE if cfg.FQ else 1.0)
                / (W_SCALE if cfg.FK else 1.0))
            nc.gpsimd.tensor_mul(probs, probs, pm_sb[:, kb, :])
            pend.append((kb, p, probs))

        for fn in v_units:
            fn()
        for p in range(cfg.NP):
            for kb in range(KB):
                att_iter(p, kb)
                if len(pend) == 3:
                    emit_ctx(pend.pop(0))
        for entry in pend:
            emit_ctx(entry)

        # ---- P3: Wo + residual + LayerNorm ---------------------------------
        for rt in range(NT):
            pso = [psum.tile([128, 512], F32, tag="ctx", bufs=4,
                             name=f"pso{ns}") for ns in range(2)]
            for p in range(cfg.NP):
                for ns in range(2):
                    nc.tensor.matmul(
                        pso[ns], ctxT_sb[:, p, rt * 128:(rt + 1) * 128],
                        wo_sb[:, p, ns * 512:ns * 512 + 512],
                        start=p == 0, stop=p == cfg.NP - 1)
            x = lnp.tile([128, D], F32, tag="x")
            for ns in range(2):
                nc.vector.tensor_add(x[:, ns * 512:ns * 512 + 512], pso[ns],
                                     res_sb[:, rt, ns * 512:ns * 512 + 512])
            fmax = math.gcd(nc.vector.BN_STATS_FMAX, D)
            nsub = D // fmax
            stats = lnp.tile([128, nsub, nc.vector.BN_STATS_DIM], F32,
                             tag="stats")
            for sg in range(nsub):
                nc.vector.bn_stats(
                    out=stats[:, sg, :],
                    in_=x.rearrange("p (a b) -> p a b", a=nsub)[:, sg, :])
            mv = lnp.tile([128, nc.vector.BN_AGGR_DIM], F32, tag="mv")
            nc.vector.bn_aggr(out=mv, in_=stats)
            sd = lnp.tile([128, 1], F32, tag="sd")
            nc.scalar.activation(out=sd, in_=mv[:, 1:2],
                                 func=mybir.ActivationFunctionType.Sqrt,
                                 bias=eps_sb, scale=1.0)
            rstd = lnp.tile([128, 1], F32, tag="rstd")
            nc.vector.reciprocal_approx_fast(rstd, sd)
            out_sb = lnp.tile([128, D], BF16, tag="out_sb")
            if cfg.G1 and cfg.B0:
                nc.vector.tensor_scalar(
                    out=out_sb, in0=x, scalar1=mv[:, 0:1], scalar2=rstd,
                    op0=mybir.AluOpType.subtract, op1=mybir.AluOpType.mult)
            else:
                y = lnp.tile([128, D], BF16, tag="y")
                nc.vector.tensor_scalar(
                    out=y, in0=x, scalar1=mv[:, 0:1], scalar2=rstd,
                    op0=mybir.AluOpType.subtract, op1=mybir.AluOpType.mult)
                if cfg.B0:
                    nc.vector.tensor_mul(out_sb, y, gamma_bc)
                elif cfg.G1:
                    nc.vector.tensor_add(out_sb, y, beta_bc)
                else:
                    yg = lnp.tile([128, D], BF16, tag="yg")
                    nc.vector.tensor_mul(yg, y, gamma_bc)
                    nc.vector.tensor_add(out_sb, yg, beta_bc)
            nc.sync.dma_start(out=out_shard[rt * 128:(rt + 1) * 128, :],
                              in_=out_sb)

    nc.compile()
    return nc


def _tile_x8(xT):
    """[D, C] f32 -> [128, D/256, 2, C] e4m3 (DoubleRow k-tile layout)."""
    d, c = xT.shape
    t = xT.reshape(d // 256, 2, 128, c).transpose(2, 0, 1, 3)
    return np.ascontiguousarray(t.astype(ml_dtypes.float8_e4m3fn))


def _tile_w(wT):
    d, o = wT.shape
    return np.ascontiguousarray(
        wT.reshape(d // 128, 128, o).transpose(1, 0, 2))


def make_in_maps(cfg: Cfg, q, k, v, Wq, Wk, Wv, Wo, gamma, beta, sen_len):
    bf = ml_dtypes.bfloat16
    q = np.asarray(q, np.float32)
    k = np.asarray(k, np.float32)
    v = np.asarray(v, np.float32)
    # fp8 weights are scaled by W_SCALE before the e4m3 cast (avoids
    # subnormals); the scale is cancelled by the exp scale (q,k) / the
    # scaled ones column (v)
    def tile_xw(xT, f8):
        return _tile_x8(xT * W_SCALE) if f8 else _tile_w(xT.astype(bf))

    wq_t = tile_xw(np.asarray(Wq, np.float32).T, cfg.FQ)
    wk_t = tile_xw(np.asarray(Wk, np.float32).T, cfg.FK)
    wv_t = tile_xw(np.asarray(Wv, np.float32).T, cfg.FV)
    wo_t = _tile_w(np.asarray(Wo, np.float32).T.astype(bf))
    g_row = np.asarray(gamma, np.float32).reshape(1, cfg.D).astype(bf)
    b_row = np.asarray(beta, np.float32).reshape(1, cfg.D).astype(bf)

    KB, KPAD, NT = cfg.KB, cfg.KPAD, cfg.NT
    per_batch = {}
    for b in range(cfg.B):
        kT_t = _tile_x8(k[b, :KPAD, :].T) if cfg.FK \
            else _tile_w(k[b, :KPAD, :].T.astype(bf))
        vT = v[b, :KPAD, :].T                     # [D, KPAD]
        if cfg.FV:
            xv = np.ascontiguousarray(
                vT.reshape(4, 2, 128, KB, 128).transpose(3, 2, 0, 1, 4)
                .astype(ml_dtypes.float8_e4m3fn))  # [KB, 128, DR, 2, 128]
        else:
            xv = np.ascontiguousarray(
                vT.astype(bf).reshape(8, 128, KB, 128)
                .transpose(2, 1, 0, 3))            # [KB, 128, DC, 128]
        per_batch[b] = (kT_t, xv)

    key_pos = np.arange(KPAD)
    in_maps = []
    for c in range(cfg.NC):
        b, qq = c // cfg.G, c % cfg.G
        tiles = [4 * j + qq for j in range(NT)]
        rows = np.concatenate(
            [np.arange(t * 128, (t + 1) * 128) for t in tiles])
        sl = int(np.asarray(sen_len)[b])

        # post-exp multiplicative mask pmask[key_p, kb, h2*512 + j*128 + f]:
        # 1 iff row (= tiles[j]*128 + f) >= key (= kb*128 + key_p) and
        # key < sen_len; identical for both h2 halves.
        rows_g = np.concatenate(
            [tiles[j] * 128 + np.arange(128) for j in range(NT)])  # [512]
        keys_g = (np.arange(KB)[:, None] * 128
                  + np.arange(128)[None, :])                       # [KB,128]
        valid = ((rows_g[None, None, :] >= keys_g[:, :, None])
                 & (keys_g[:, :, None] < sl))                      # [KB,128,512]
        pm = np.broadcast_to(
            valid.transpose(1, 0, 2)[:, :, None, :],
            (128, KB, 2, 512)).reshape(128, KB, 1024)
        pm = np.ascontiguousarray(pm.astype(ml_dtypes.bfloat16))

        xq_h = _tile_x8(q[b][rows, :].T) if cfg.FQ \
            else _tile_w(q[b][rows, :].T.astype(bf))
        res = np.ascontiguousarray(
            q[b][rows, :].reshape(NT, 128, cfg.D)
            .transpose(1, 0, 2).astype(bf))
        kT_t, xv_t = per_batch[b]
        in_maps.append({
            "xq": xq_h, "xk": kT_t, "xv": xv_t,
            "wqT": wq_t, "wkT": wk_t, "wvT": wv_t, "woT": wo_t,
            "pmask": pm, "resid": res,
            "gamma": g_row, "beta": b_row,
        })
    return in_maps


def assemble_output(cfg: Cfg, results):
    out = np.empty((cfg.B, cfg.S, cfg.D), np.float32)
    for c in range(cfg.NC):
        b, qq = c // cfg.G, c % cfg.G
        shard = results[c]["out_shard"].astype(np.float32)
        for j in range(cfg.NT):
            t = 4 * j + qq
            out[b, t * 128:(t + 1) * 128, :] = shard[j * 128:(j + 1) * 128]
    return out


_PROGRAM_CACHE = {}


def _get_program(cfg: Cfg):
    key = (cfg.B, cfg.S, cfg.D, cfg.H, cfg.dh, cfg.KB, cfg.G1, cfg.B0,
           cfg.FQ, cfg.FK, cfg.FV)
    if key not in _PROGRAM_CACHE:
        _PROGRAM_CACHE[key] = build_program(cfg)
    return _PROGRAM_CACHE[key]


def run(cfg: Cfg, inputs: dict, trace: bool = False):
    cfg.G1 = bool(np.all(np.asarray(inputs["gamma"]) == 1.0))
    cfg.B0 = bool(np.all(np.asarray(inputs["beta"]) == 0.0))
    nc = _get_program(cfg)
    in_maps = make_in_maps(cfg, **inputs)
    res = run_bass_kernel_spmd(nc, in_maps, core_ids=list(range(cfg.NC)),
                               trace=trace)
    return assemble_output(cfg, res.results), res


def kernel(**inputs) -> np.ndarray:
    kmax = int(np.max(inputs["sen_len"]))
    cfg = Cfg(B=2, S=2048, D=1024, H=16, dh=64, kmax=kmax)
    out, _ = run(cfg, inputs)
    return out


# revision 96
# speedup vs baseline: 1.8525x; 1.6764x over previous
"""Multi-head attention (QKV projections + causal/padded softmax attention +
output projection + residual + LayerNorm) as a Bass/Tile kernel on 8 Trainium2
cores — NO collectives.

Sharding: rows (sequence) are sharded across cores; every core computes ALL 16
heads for its own 512 rows end-to-end, so no cross-core communication is ever
needed.  Core c handles batch b = c//4 and the four 128-row tiles
t_j = 4*j + (c%4), j = 0..3.  The price is that each 4-core batch group
re-computes the batch's K/V projections (up to kmax keys) redundantly; that
costs ~25us of PE but saves the ~100us collective chain (CC-stream barrier +
2 AllToAlls) the head-sharded variant pays in this environment.

SPMD trick for the causal mask: the program is identical on all cores, but
the causal-diagonal position is core-dependent (rows differ per core).  ALL
masking (key padding, whole chunks above the diagonal, and the in-diagonal
triangle) is therefore one multiplicative 0/1 bf16 mask per key-chunk,
precomputed PER CORE on the host and applied to the probs AFTER the exp on
the (otherwise idle) GpSimd engine — nothing mask-related ever touches the
scores->exp critical chain, and exp of unmasked scores is safe (|s|/8 is a
few units at most).  V is augmented with a ones column so row 64 of the ctx
psum accumulates the softmax denominators for free.

Attention is pair-major: per (head-pair, key-chunk) one 512-row score matmul
per head half into separate psum banks (a hard HW constraint: matmuls whose
stationary sits at different partition bases — PE row-tile 0 vs 64 — must
not write the same PSUM bank, or the NEFF dies at runtime), one [128,1024]
exp, and two 512-row ctx matmuls accumulated over chunks.  512-wide moving
operands matter: matmul wall time is ~(moving x 0.42ns + ~145ns overhead +
LDWEIGHTS), so few big matmuls beat many small ones.

All engines are load-balanced: PE ~120us of matmul, scalar exp ~48us + half
the psum->sbuf copies, DVE the other copies + denominators + LayerNorm,
GpSimd the masks/broadcasts.  Weights stream per-dc-chunk on the scalar DMA
queue in first-use order (wk, wq, wv, then wo late); activations on the sync
queue; masks/residual/LN constants behind them.  fp8(e4m3) DoubleRow
projections were tried and measured 0.025 max-rel-err on hardware (2x the
simulator estimate) against the 0.02 gate, so everything stays bf16.
"""

import math
from contextlib import ExitStack

import numpy as np
import ml_dtypes

import concourse.mybir as mybir
import concourse.tile as tile
from concourse import bacc
from concourse.bass_utils import run_bass_kernel_spmd

BF16 = mybir.dt.bfloat16
F32 = mybir.dt.float32
FP8 = mybir.dt.float8e4
W_SCALE = 8.0  # host scales W and x->e4m3; folded back via exp scale / ones

NEG_INF = -1e9
LN_EPS = 1e-6


class Cfg:
    def __init__(self, B=2, S=2048, D=1024, H=16, dh=64, kmax=None):
        self.B, self.S, self.D, self.H, self.dh = B, S, D, H, dh
        self.kmax = S if kmax is None else max(1, min(int(kmax), S))
        self.NC = 8                       # cores
        self.G = 4                        # cores per batch group
        self.RPC = S // self.G            # rows per core (512)
        self.NT = self.RPC // 128         # row-tiles per core (4)
        self.DC = D // 128                # contraction chunks (8)
        self.NP = H // 2                  # head pairs (8)
        self.KB = -(-self.kmax // 128)    # key chunks actually needed
        self.KPAD = self.KB * 128
        # slot j covers row tile 4*j+q (q = core quarter); the static chunk
        # cap must cover the deepest core (q=3)
        self.caps = [min(4 * j + 4, self.KB) for j in range(self.NT)]
        # (j, kb) positions where ANY core's causal diagonal can fall
        self.POS = [(j, kb) for j in range(self.NT)
                    for kb in range(self.caps[j])
                    if 4 * j <= kb <= 4 * j + 3]
        # runtime-detected LN specializations
        self.G1 = False
        self.B0 = False
        # per-projection fp8 (DoubleRow) selection.  All False: e4m3
        # projections measured 0.025 max-rel-err on HW (2x the simulator's
        # 0.013) against the 0.02 gate — not worth the risk for ~15us.
        self.FQ = False
        self.FK = False
        self.FV = False


def build_program(cfg: Cfg):
    nc = bacc.Bacc("TRN2", target_bir_lowering=False, debug=False,
                   num_devices=cfg.NC)

    D, dh = cfg.D, cfg.dh
    KB, KPAD, RPC, NT = cfg.KB, cfg.KPAD, cfg.RPC, cfg.NT

    # Selected projections are pre-quantized to e4m3 on the host (weights
    # scaled by W_SCALE) and consumed by DoubleRow matmuls: 256-deep
    # contraction per instruction at 0.5 cycles/row — half the HBM bytes,
    # half the PE instructions, and half the cycles of the bf16 path.
    DR = cfg.DC // 2

    def x_in(name, cols, f8):
        if f8:
            return nc.dram_tensor(name, [128, DR, 2, cols], FP8,
                                  kind="ExternalInput").ap()
        return nc.dram_tensor(name, [128, cfg.DC, cols], BF16,
                              kind="ExternalInput").ap()

    xq = x_in("xq", RPC, cfg.FQ)
    xk = x_in("xk", KPAD, cfg.FK)
    wqT = x_in("wqT", D, cfg.FQ)
    wkT = x_in("wkT", D, cfg.FK)
    wvT = x_in("wvT", D, cfg.FV)
    if cfg.FV:
        xv = nc.dram_tensor("xv", [KB, 128, DR, 2, 128], FP8,
                            kind="ExternalInput").ap()
    else:
        xv = nc.dram_tensor("xv", [KB, 128, cfg.DC, 128], BF16,
                            kind="ExternalInput").ap()
    woT = nc.dram_tensor("woT", [128, cfg.DC, D], BF16,
                         kind="ExternalInput").ap()
    # per-core post-exp mask: pmask[key, kb, h2*512 + j*128 + f] in {0,1}
    # covers pad (key >= sen_len), causal (row < key), and above-diagonal
    # chunks, identical for every head; the 512-row block is stored twice
    # (h2 = 0/1) so one [128, 1024] multiply masks a whole pair tile.
    pmask = nc.dram_tensor("pmask", [128, KB, 1024], BF16,
                           kind="ExternalInput").ap()
    resid = nc.dram_tensor("resid", [128, NT, D], BF16,
                           kind="ExternalInput").ap()
    gamma = nc.dram_tensor("gamma", [1, D], BF16, kind="ExternalInput").ap()
    beta = nc.dram_tensor("beta", [1, D], BF16, kind="ExternalInput").ap()
    out_shard = nc.dram_tensor("out_shard", [RPC, D], BF16,
                               kind="ExternalOutput").ap()

    with tile.TileContext(nc) as tc, ExitStack() as ctx:
        consts = ctx.enter_context(tc.tile_pool(name="consts", bufs=1))
        xin = ctx.enter_context(tc.tile_pool(name="xin", bufs=1))
        proj = ctx.enter_context(tc.tile_pool(name="proj", bufs=1))
        att = ctx.enter_context(tc.tile_pool(name="att", bufs=2))
        small = ctx.enter_context(tc.tile_pool(name="small", bufs=2))
        lnp = ctx.enter_context(tc.tile_pool(name="lnp", bufs=2))
        psum = ctx.enter_context(
            tc.tile_pool(name="psum", bufs=1, space="PSUM"))

        # ---- constants -----------------------------------------------------
        # weights ride the scalar engine's DMA queue (parallel to the sync
        # queue that streams activations); per-dc chunks so the first
        # accumulation chain can start after 1/8 of the weight transfer.
        def w_tile(name, f8):
            if f8:
                return consts.tile([128, DR, 2, D], FP8, name=name)
            return consts.tile([128, cfg.DC, D], BF16, name=name)

        wk_sb = w_tile("wk_sb", cfg.FK)
        wq_sb = w_tile("wq_sb", cfg.FQ)
        wv_sb = w_tile("wv_sb", cfg.FV)
        for w_sb, w_dram in ((wk_sb, wkT), (wq_sb, wqT), (wv_sb, wvT)):
            for i in range(w_sb.shape[1]):
                nc.scalar.dma_start(out=w_sb[:, i], in_=w_dram[:, i])



        # P3 constants (loaded later on the sync queue, behind the inputs)
        gamma_bc = beta_bc = None
        if not (cfg.G1 and cfg.B0):
            g_row = consts.tile([1, D], BF16)
            b_row = consts.tile([1, D], BF16)
            gamma_bc = consts.tile([128, D], BF16)
            beta_bc = consts.tile([128, D], BF16)
        eps_sb = consts.tile([128, 1], F32)
        nc.vector.memset(eps_sb, LN_EPS)
        res_sb = consts.tile([128, NT, D], BF16)

        # ---- P1: projections ----------------------------------------------
        khT_sb = proj.tile([128, cfg.NP, KPAD], BF16)
        qhT_sb = proj.tile([128, cfg.NP, RPC], BF16)
        vh_sb = proj.tile([128, KB, cfg.H, dh + 1], BF16)
        # the ones column carries W_SCALE so ctx/den cancels the vh scale
        nc.gpsimd.memset(vh_sb[:, :, :, dh:dh + 1],
                         W_SCALE if cfg.FV else 1.0)

        # per-chunk DMAs (contiguous per partition): the first K matmul
        # only waits on chunk 0, not the whole transfer
        def x_tile(name, cols, f8):
            if f8:
                t = xin.tile([128, DR, 2, cols], FP8, name=name)
            else:
                t = xin.tile([128, cfg.DC, cols], BF16, name=name)
            return t

        xk_sb = x_tile("xk_sb", KPAD, cfg.FK)
        for i in range(xk_sb.shape[1]):
            nc.sync.dma_start(out=xk_sb[:, i], in_=xk[:, i])
        xq_sb = x_tile("xq_sb", RPC, cfg.FQ)
        for i in range(xq_sb.shape[1]):
            nc.sync.dma_start(out=xq_sb[:, i], in_=xq[:, i])
        xv_sb = []
        for kb in range(KB):
            if cfg.FV:
                t = xin.tile([128, DR, 2, 128], FP8, tag="xv", bufs=KB,
                             name=f"xv{kb}")
            else:
                t = xin.tile([128, cfg.DC, 128], BF16, tag="xv", bufs=KB,
                             name=f"xv{kb}")
            nc.sync.dma_start(out=t, in_=xv[kb])
            xv_sb.append(t)

        copy_flip = [0]

        def pcopy(out_ap, in_ap):
            # alternate psum->sbuf copies between DVE and the scalar engine
            eng = nc.vector if copy_flip[0] % 2 == 0 else nc.scalar
            copy_flip[0] += 1
            if eng is nc.vector:
                eng.tensor_copy(out=out_ap, in_=in_ap)
            else:
                eng.copy(out=out_ap, in_=in_ap)

        DRM = mybir.MatmulPerfMode.DoubleRow

        def xw_proj(x_sb, w_sb, out_sb, width, f8):
            # fp8: lhsT = w [128, 2, 128] (two stacked k-tiles), rhs = x
            # [128, 2, cols]: 256-deep contraction per DoubleRow matmul
            nch = DR if f8 else cfg.DC
            for p in range(cfg.NP):
                for c0 in range(0, width, 512):
                    cw = min(512, width - c0)
                    ps = psum.tile([128, cw], F32, tag="ctx", bufs=4,
                                   name="ps_proj")
                    for i in range(nch):
                        if f8:
                            nc.tensor.matmul(
                                ps, w_sb[:, i, :, p * 128:(p + 1) * 128],
                                x_sb[:, i, :, c0:c0 + cw],
                                start=i == 0, stop=i == nch - 1,
                                perf_mode=DRM)
                        else:
                            nc.tensor.matmul(
                                ps, w_sb[:, i, p * 128:(p + 1) * 128],
                                x_sb[:, i, c0:c0 + cw],
                                start=i == 0, stop=i == nch - 1)
                    pcopy(out_sb[:, p, c0:c0 + cw], ps)

        # K first (scores need it first), then Q.  The V projections are
        # deferred: one V unit is emitted after each of the first two
        # pairs' score/exp iterations, so the scalar engine's softmax
        # backlog overlaps the PE's V work.  V uses the "ctx" psum ring,
        # which holds no live ctx tiles yet (they allocate lazily in
        # emit_ctx, after V), so ring reuse stays acyclic.
        xw_proj(xk_sb, wk_sb, khT_sb, KPAD, cfg.FK)
        xw_proj(xq_sb, wq_sb, qhT_sb, RPC, cfg.FQ)

        nchv = DR if cfg.FV else cfg.DC

        def v_unit(kb, half):
            def run():
                ps = psum.tile([128, 512], F32, tag="ctx", bufs=4,
                               name="ps_v")
                for i in range(nchv):
                    if cfg.FV:
                        nc.tensor.matmul(
                            ps, xv_sb[kb][:, i],
                            wv_sb[:, i, :, half * 512:half * 512 + 512],
                            start=i == 0, stop=i == nchv - 1,
                            perf_mode=DRM)
                    else:
                        nc.tensor.matmul(
                            ps, xv_sb[kb][:, i],
                            wv_sb[:, i, half * 512:half * 512 + 512],
                            start=i == 0, stop=i == nchv - 1)
                pcopy(
                    vh_sb[:, kb, 8 * half:8 * half + 8, 0:dh],
                    ps.rearrange("p (h e) -> p h e", e=dh))
            return run

        v_units = [v_unit(kb, half) for kb in range(KB) for half in range(2)]

        # late transfers, in need order: pmask (first diag ~30us), then wo
        # (needed ~130us), then the LN constants — all behind the
        # critical-path weight/input streams
        pm_sb = consts.tile([128, KB, 1024], BF16)
        nc.sync.dma_start(out=pm_sb, in_=pmask)
        wo_sb = consts.tile([128, cfg.DC, D], BF16, name="wo_sb")
        for dc in range(cfg.DC):
            nc.scalar.dma_start(out=wo_sb[:, dc, :], in_=woT[:, dc, :])
        nc.sync.dma_start(out=res_sb, in_=resid)
        if gamma_bc is not None:
            nc.sync.dma_start(out=g_row, in_=gamma)
            nc.sync.dma_start(out=b_row, in_=beta)
            nc.gpsimd.partition_broadcast(gamma_bc, g_row)
            nc.gpsimd.partition_broadcast(beta_bc, b_row)

        # ---- P2: attention (pair-major, all 512 rows per tile) -------------
        ctxT_sb = proj.tile([128, cfg.NP, RPC], BF16)

        def divide(p, ctx_ps):
            # ctx rows 0..63 / den row 64; stage ctxT = ctx/den for Wo
            for h2 in range(2):
                den = small.tile([1, 512], F32, tag=f"den{h2}", bufs=2,
                                 name=f"den{h2}")
                nc.vector.tensor_copy(out=den, in_=ctx_ps[h2][dh:dh + 1, :])
                rec = small.tile([1, 512], F32, tag=f"rec{h2}", bufs=2,
                                 name=f"rec{h2}")
                nc.vector.reciprocal_approx_fast(rec, den)
                rbc = small.tile([64, 512], F32, tag=f"rbc{h2}", bufs=2,
                                 name=f"rbc{h2}")
                nc.gpsimd.partition_broadcast(rbc, rec)
                nc.vector.tensor_mul(
                    ctxT_sb[64 * h2:64 * h2 + 64, p, :],
                    ctx_ps[h2][0:dh, :], rbc)

        ctx_tiles = {}

        def emit_ctx(entry):
            kb, p, probs = entry
            if p not in ctx_tiles:
                ctx_tiles[p] = [psum.tile([dh + 1, 512], F32, tag="ctx",
                                          bufs=4, name=f"ctx{h2}")
                                for h2 in range(2)]
            ctx_ps = ctx_tiles[p]
            for h2 in range(2):
                nc.tensor.matmul(
                    ctx_ps[h2], vh_sb[:, kb, 2 * p + h2, :],
                    probs[:, h2 * 512:(h2 + 1) * 512],
                    start=kb == 0, stop=kb == KB - 1)
            if kb == KB - 1:
                divide(p, ctx_ps)
                del ctx_tiles[p]

        pend = []

        def att_iter(p, kb):
            sc = psum.tile([128, 1024], F32, tag="sc", bufs=2, name="sc")
            for h2 in range(2):
                lo = 64 * h2
                nc.tensor.matmul(
                    sc[:, h2 * 512:(h2 + 1) * 512],
                    khT_sb[lo:lo + 64, p, kb * 128:(kb + 1) * 128],
                    qhT_sb[lo:lo + 64, p, :],
                    start=True, stop=True)
            probs = att.tile([128, 1024], BF16, tag="pr", bufs=12,
                             name="probs")
            nc.scalar.activation(
                out=probs, in_=sc,
                func=mybir.ActivationFunctionType.Exp,
                scale=1.0 / math.sqrt(dh)
                / (W_SCALE if cfg.FQ else 1.0)
                / (W_SCALE if cfg.FK else 1.0))
            nc.vector.tensor_mul(probs, probs, pm_sb[:, kb, :])
            pend.append((kb, p, probs))

        for fn in v_units:
            fn()
        for p in range(cfg.NP):
            for kb in range(KB):
                att_iter(p, kb)
                if len(pend) == 3:
                    emit_ctx(pend.pop(0))
        for entry in pend:
            emit_ctx(entry)

        # ---- P3: Wo + residual + LayerNorm ---------------------------------
        for rt in range(NT):
            pso = [psum.tile([128, 512], F32, tag="ctx", bufs=4,
                             name=f"pso{ns}") for ns in range(2)]
            for p in range(cfg.NP):
                for ns in range(2):
                    nc.tensor.matmul(
                        pso[ns], ctxT_sb[:, p, rt * 128:(rt + 1) * 128],
                        wo_sb[:, p, ns * 512:ns * 512 + 512],
                        start=p == 0, stop=p == cfg.NP - 1)
            x = lnp.tile([128, D], F32, tag="x")
            for ns in range(2):
                nc.vector.tensor_add(x[:, ns * 512:ns * 512 + 512], pso[ns],
                                     res_sb[:, rt, ns * 512:ns * 512 + 512])
            fmax = math.gcd(nc.vector.BN_STATS_FMAX, D)
            nsub = D // fmax
            stats = lnp.tile([128, nsub, nc.vector.BN_STATS_DIM], F32,
                             tag="stats")
            for sg in range(nsub):
                nc.vector.bn_stats(
                    out=stats[:, sg, :],
                    in_=x.rearrange("p (a b) -> p a b", a=nsub)[:, sg, :])
            mv = lnp.tile([128, nc.vector.BN_AGGR_DIM], F32, tag="mv")
            nc.vector.bn_aggr(out=mv, in_=stats)
            sd = lnp.tile([128, 1], F32, tag="sd")
            nc.scalar.activation(out=sd, in_=mv[:, 1:2],
                                 func=mybir.ActivationFunctionType.Sqrt,
                                 bias=eps_sb, scale=1.0)
            rstd = lnp.tile([128, 1], F32, tag="rstd")
            nc.vector.reciprocal_approx_fast(rstd, sd)
            out_sb = lnp.tile([128, D], BF16, tag="out_sb")
            if cfg.G1 and cfg.B0:
                nc.vector.tensor_scalar(
                    out=out_sb, in0=x, scalar1=mv[:, 0:1], scalar2=rstd,
                    op0=mybir.AluOpType.subtract, op1=mybir.AluOpType.mult)
            else:
                y = lnp.tile([128, D], BF16, tag="y")
                nc.vector.tensor_scalar(
                    out=y, in0=x, scalar1=mv[:, 0:1], scalar2=rstd,
                    op0=mybir.AluOpType.subtract, op1=mybir.AluOpType.mult)
                if cfg.B0:
                    nc.vector.tensor_mul(out_sb, y, gamma_bc)
                elif cfg.G1:
                    nc.vector.tensor_add(out_sb, y, beta_bc)
                else:
                    yg = lnp.tile([128, D], BF16, tag="yg")
                    nc.vector.tensor_mul(yg, y, gamma_bc)
                    nc.vector.tensor_add(out_sb, yg, beta_bc)
            nc.sync.dma_start(out=out_shard[rt * 128:(rt + 1) * 128, :],
                              in_=out_sb)

    nc.compile()
    return nc


def _tile_x8(xT):
    """[D, C] f32 -> [128, D/256, 2, C] e4m3 (DoubleRow k-tile layout)."""
    d, c = xT.shape
    t = xT.reshape(d // 256, 2, 128, c).transpose(2, 0, 1, 3)
    return np.ascontiguousarray(t.astype(ml_dtypes.float8_e4m3fn))


def _tile_w(wT):
    d, o = wT.shape
    return np.ascontiguousarray(
        wT.reshape(d // 128, 128, o).transpose(1, 0, 2))


def make_in_maps(cfg: Cfg, q, k, v, Wq, Wk, Wv, Wo, gamma, beta, sen_len):
    bf = ml_dtypes.bfloat16
    q = np.asarray(q, np.float32)
    k = np.asarray(k, np.float32)
    v = np.asarray(v, np.float32)
    # fp8 weights are scaled by W_SCALE before the e4m3 cast (avoids
    # subnormals); the scale is cancelled by the exp scale (q,k) / the
    # scaled ones column (v)
    def tile_xw(xT, f8):
        return _tile_x8(xT * W_SCALE) if f8 else _tile_w(xT.astype(bf))

    wq_t = tile_xw(np.asarray(Wq, np.float32).T, cfg.FQ)
    wk_t = tile_xw(np.asarray(Wk, np.float32).T, cfg.FK)
    wv_t = tile_xw(np.asarray(Wv, np.float32).T, cfg.FV)
    wo_t = _tile_w(np.asarray(Wo, np.float32).T.astype(bf))
    g_row = np.asarray(gamma, np.float32).reshape(1, cfg.D).astype(bf)
    b_row = np.asarray(beta, np.float32).reshape(1, cfg.D).astype(bf)

    KB, KPAD, NT = cfg.KB, cfg.KPAD, cfg.NT
    per_batch = {}
    for b in range(cfg.B):
        kT_t = _tile_x8(k[b, :KPAD, :].T) if cfg.FK \
            else _tile_w(k[b, :KPAD, :].T.astype(bf))
        vT = v[b, :KPAD, :].T                     # [D, KPAD]
        if cfg.FV:
            xv = np.ascontiguousarray(
                vT.reshape(4, 2, 128, KB, 128).transpose(3, 2, 0, 1, 4)
                .astype(ml_dtypes.float8_e4m3fn))  # [KB, 128, DR, 2, 128]
        else:
            xv = np.ascontiguousarray(
                vT.astype(bf).reshape(8, 128, KB, 128)
                .transpose(2, 1, 0, 3))            # [KB, 128, DC, 128]
        per_batch[b] = (kT_t, xv)

    key_pos = np.arange(KPAD)
    in_maps = []
    for c in range(cfg.NC):
        b, qq = c // cfg.G, c % cfg.G
        tiles = [4 * j + qq for j in range(NT)]
        rows = np.concatenate(
            [np.arange(t * 128, (t + 1) * 128) for t in tiles])
        sl = int(np.asarray(sen_len)[b])

        # post-exp multiplicative mask pmask[key_p, kb, h2*512 + j*128 + f]:
        # 1 iff row (= tiles[j]*128 + f) >= key (= kb*128 + key_p) and
        # key < sen_len; identical for both h2 halves.
        rows_g = np.concatenate(
            [tiles[j] * 128 + np.arange(128) for j in range(NT)])  # [512]
        keys_g = (np.arange(KB)[:, None] * 128
                  + np.arange(128)[None, :])                       # [KB,128]
        valid = ((rows_g[None, None, :] >= keys_g[:, :, None])
                 & (keys_g[:, :, None] < sl))                      # [KB,128,512]
        pm = np.broadcast_to(
            valid.transpose(1, 0, 2)[:, :, None, :],
            (128, KB, 2, 512)).reshape(128, KB, 1024)
        pm = np.ascontiguousarray(pm.astype(ml_dtypes.bfloat16))

        xq_h = _tile_x8(q[b][rows, :].T) if cfg.FQ \
            else _tile_w(q[b][rows, :].T.astype(bf))
        res = np.ascontiguousarray(
            q[b][rows, :].reshape(NT, 128, cfg.D)
            .transpose(1, 0, 2).astype(bf))
        kT_t, xv_t = per_batch[b]
        in_maps.append({
            "xq": xq_h, "xk": kT_t, "xv": xv_t,
            "wqT": wq_t, "wkT": wk_t, "wvT": wv_t, "woT": wo_t,
            "pmask": pm, "resid": res,
            "gamma": g_row, "beta": b_row,
        })
    return in_maps


def assemble_output(cfg: Cfg, results):
    out = np.empty((cfg.B, cfg.S, cfg.D), np.float32)
    for c in range(cfg.NC):
        b, qq = c // cfg.G, c % cfg.G
        shard = results[c]["out_shard"].astype(np.float32)
        for j in range(cfg.NT):
            t = 4 * j + qq
            out[b, t * 128:(t + 1) * 128, :] = shard[j * 128:(j + 1) * 128]
    return out


_PROGRAM_CACHE = {}


def _get_program(cfg: Cfg):
    key = (cfg.B, cfg.S, cfg.D, cfg.H, cfg.dh, cfg.KB, cfg.G1, cfg.B0,
           cfg.FQ, cfg.FK, cfg.FV)
    if key not in _PROGRAM_CACHE:
        _PROGRAM_CACHE[key] = build_program(cfg)
    return _PROGRAM_CACHE[key]


def run(cfg: Cfg, inputs: dict, trace: bool = False):
    cfg.G1 = bool(np.all(np.asarray(inputs["gamma"]) == 1.0))
    cfg.B0 = bool(np.all(np.asarray(inputs["beta"]) == 0.0))
    nc = _get_program(cfg)
    in_maps = make_in_maps(cfg, **inputs)
    res = run_bass_kernel_spmd(nc, in_maps, core_ids=list(range(cfg.NC)),
                               trace=trace)
    return assemble_output(cfg, res.results), res


def kernel(**inputs) -> np.ndarray:
    kmax = int(np.max(inputs["sen_len"]))
    cfg = Cfg(B=2, S=2048, D=1024, H=16, dh=64, kmax=kmax)
    out, _ = run(cfg, inputs)
    return out


# revision 97
# speedup vs baseline: 1.9301x; 1.0419x over previous
"""Multi-head attention (QKV projections + causal/padded softmax attention +
output projection + residual + LayerNorm) as a Bass/Tile kernel on 8 Trainium2
cores — NO collectives.

Sharding: rows (sequence) are sharded across cores; every core computes ALL 16
heads for its own 512 rows end-to-end, so no cross-core communication is ever
needed.  Core c handles batch b = c//4 and the four 128-row tiles
t_j = 4*j + (c%4), j = 0..3.  The price is that each 4-core batch group
re-computes the batch's K/V projections (up to kmax keys) redundantly; that
costs ~25us of PE but saves the ~100us collective chain (CC-stream barrier +
2 AllToAlls) the head-sharded variant pays in this environment.

SPMD trick for the causal mask: the program is identical on all cores, but
the causal-diagonal position is core-dependent (rows differ per core).  ALL
masking (key padding, whole chunks above the diagonal, and the in-diagonal
triangle) is therefore one multiplicative 0/1 bf16 mask per key-chunk,
precomputed PER CORE on the host and applied to the probs AFTER the exp on
the (otherwise idle) GpSimd engine — nothing mask-related ever touches the
scores->exp critical chain, and exp of unmasked scores is safe (|s|/8 is a
few units at most).  V is augmented with a ones column so row 64 of the ctx
psum accumulates the softmax denominators for free.

Attention is pair-major: per (head-pair, key-chunk) one 512-row score matmul
per head half into separate psum banks (a hard HW constraint: matmuls whose
stationary sits at different partition bases — PE row-tile 0 vs 64 — must
not write the same PSUM bank, or the NEFF dies at runtime), one [128,1024]
exp, and two 512-row ctx matmuls accumulated over chunks.  512-wide moving
operands matter: matmul wall time is ~(moving x 0.42ns + ~145ns overhead +
LDWEIGHTS), so few big matmuls beat many small ones.

All engines are load-balanced: PE ~120us of matmul, scalar exp ~48us + half
the psum->sbuf copies, DVE the other copies + denominators + LayerNorm,
GpSimd the masks/broadcasts.  Weights stream per-dc-chunk on the scalar DMA
queue in first-use order (wk, wq, wv, then wo late); activations on the sync
queue; masks/residual/LN constants behind them.  fp8(e4m3) DoubleRow
projections were tried and measured 0.025 max-rel-err on hardware (2x the
simulator estimate) against the 0.02 gate, so everything stays bf16.
"""

import math
from contextlib import ExitStack

import numpy as np
import ml_dtypes

import concourse.mybir as mybir
import concourse.tile as tile
from concourse import bacc
from concourse.bass_utils import run_bass_kernel_spmd

BF16 = mybir.dt.bfloat16
F32 = mybir.dt.float32
FP8 = mybir.dt.float8e4
W_SCALE = 8.0  # host scales W and x->e4m3; folded back via exp scale / ones

NEG_INF = -1e9
LN_EPS = 1e-6


class Cfg:
    def __init__(self, B=2, S=2048, D=1024, H=16, dh=64, kmax=None):
        self.B, self.S, self.D, self.H, self.dh = B, S, D, H, dh
        self.kmax = S if kmax is None else max(1, min(int(kmax), S))
        self.NC = 8                       # cores
        self.G = 4                        # cores per batch group
        self.RPC = S // self.G            # rows per core (512)
        self.NT = self.RPC // 128         # row-tiles per core (4)
        self.DC = D // 128                # contraction chunks (8)
        self.NP = H // 2                  # head pairs (8)
        self.KB = -(-self.kmax // 128)    # key chunks actually needed
        self.KPAD = self.KB * 128
        # slot j covers row tile 4*j+q (q = core quarter); the static chunk
        # cap must cover the deepest core (q=3)
        self.caps = [min(4 * j + 4, self.KB) for j in range(self.NT)]
        # (j, kb) positions where ANY core's causal diagonal can fall
        self.POS = [(j, kb) for j in range(self.NT)
                    for kb in range(self.caps[j])
                    if 4 * j <= kb <= 4 * j + 3]
        # runtime-detected LN specializations
        self.G1 = False
        self.B0 = False
        # per-projection fp8 (DoubleRow) selection.  All False: e4m3
        # projections measured 0.025 max-rel-err on HW (2x the simulator's
        # 0.013) against the 0.02 gate — not worth the risk for ~15us.
        self.FQ = False
        self.FK = False
        self.FV = False


def build_program(cfg: Cfg):
    nc = bacc.Bacc("TRN2", target_bir_lowering=False, debug=False,
                   num_devices=cfg.NC)

    D, dh = cfg.D, cfg.dh
    KB, KPAD, RPC, NT = cfg.KB, cfg.KPAD, cfg.RPC, cfg.NT

    # Selected projections are pre-quantized to e4m3 on the host (weights
    # scaled by W_SCALE) and consumed by DoubleRow matmuls: 256-deep
    # contraction per instruction at 0.5 cycles/row — half the HBM bytes,
    # half the PE instructions, and half the cycles of the bf16 path.
    DR = cfg.DC // 2

    def x_in(name, cols, f8):
        if f8:
            return nc.dram_tensor(name, [128, DR, 2, cols], FP8,
                                  kind="ExternalInput").ap()
        return nc.dram_tensor(name, [128, cfg.DC, cols], BF16,
                              kind="ExternalInput").ap()

    xq = x_in("xq", RPC, cfg.FQ)
    xk = x_in("xk", KPAD, cfg.FK)
    wqT = x_in("wqT", D, cfg.FQ)
    wkT = x_in("wkT", D, cfg.FK)
    wvT = x_in("wvT", D, cfg.FV)
    if cfg.FV:
        xv = nc.dram_tensor("xv", [KB, 128, DR, 2, 128], FP8,
                            kind="ExternalInput").ap()
    else:
        xv = nc.dram_tensor("xv", [KB, 128, cfg.DC, 128], BF16,
                            kind="ExternalInput").ap()
    woT = nc.dram_tensor("woT", [128, cfg.DC, D], BF16,
                         kind="ExternalInput").ap()
    # per-core post-exp mask: pmask[key, kb, h2*512 + j*128 + f] in {0,1}
    # covers pad (key >= sen_len), causal (row < key), and above-diagonal
    # chunks, identical for every head; the 512-row block is stored twice
    # (h2 = 0/1) so one [128, 1024] multiply masks a whole pair tile.
    pmask = nc.dram_tensor("pmask", [128, KB, 1024], BF16,
                           kind="ExternalInput").ap()
    resid = nc.dram_tensor("resid", [128, NT, D], BF16,
                           kind="ExternalInput").ap()
    gamma = nc.dram_tensor("gamma", [1, D], BF16, kind="ExternalInput").ap()
    beta = nc.dram_tensor("beta", [1, D], BF16, kind="ExternalInput").ap()
    out_shard = nc.dram_tensor("out_shard", [RPC, D], BF16,
                               kind="ExternalOutput").ap()

    with tile.TileContext(nc) as tc, ExitStack() as ctx:
        consts = ctx.enter_context(tc.tile_pool(name="consts", bufs=1))
        xin = ctx.enter_context(tc.tile_pool(name="xin", bufs=1))
        proj = ctx.enter_context(tc.tile_pool(name="proj", bufs=1))
        att = ctx.enter_context(tc.tile_pool(name="att", bufs=2))
        small = ctx.enter_context(tc.tile_pool(name="small", bufs=2))
        lnp = ctx.enter_context(tc.tile_pool(name="lnp", bufs=2))
        psum = ctx.enter_context(
            tc.tile_pool(name="psum", bufs=1, space="PSUM"))

        # ---- constants -----------------------------------------------------
        # weights ride the scalar engine's DMA queue (parallel to the sync
        # queue that streams activations); per-dc chunks so the first
        # accumulation chain can start after 1/8 of the weight transfer.
        def w_tile(name, f8):
            if f8:
                return consts.tile([128, DR, 2, D], FP8, name=name)
            return consts.tile([128, cfg.DC, D], BF16, name=name)

        wk_sb = w_tile("wk_sb", cfg.FK)
        wq_sb = w_tile("wq_sb", cfg.FQ)
        wv_sb = w_tile("wv_sb", cfg.FV)
        for w_sb, w_dram in ((wk_sb, wkT), (wq_sb, wqT), (wv_sb, wvT)):
            for i in range(w_sb.shape[1]):
                nc.scalar.dma_start(out=w_sb[:, i], in_=w_dram[:, i])



        # P3 constants (loaded later on the sync queue, behind the inputs)
        gamma_bc = beta_bc = None
        if not (cfg.G1 and cfg.B0):
            g_row = consts.tile([1, D], BF16)
            b_row = consts.tile([1, D], BF16)
            gamma_bc = consts.tile([128, D], BF16)
            beta_bc = consts.tile([128, D], BF16)
        eps_sb = consts.tile([128, 1], F32)
        nc.vector.memset(eps_sb, LN_EPS)
        res_sb = consts.tile([128, NT, D], BF16)

        # ---- P1: projections ----------------------------------------------
        khT_sb = proj.tile([128, cfg.NP, KPAD], BF16)
        qhT_sb = proj.tile([128, cfg.NP, RPC], BF16)
        vh_sb = proj.tile([128, KB, cfg.H, dh + 1], BF16)
        # the ones column carries W_SCALE so ctx/den cancels the vh scale
        nc.gpsimd.memset(vh_sb[:, :, :, dh:dh + 1],
                         W_SCALE if cfg.FV else 1.0)

        # per-chunk DMAs (contiguous per partition): the first K matmul
        # only waits on chunk 0, not the whole transfer
        def x_tile(name, cols, f8):
            if f8:
                t = xin.tile([128, DR, 2, cols], FP8, name=name)
            else:
                t = xin.tile([128, cfg.DC, cols], BF16, name=name)
            return t

        xk_sb = x_tile("xk_sb", KPAD, cfg.FK)
        for i in range(xk_sb.shape[1]):
            nc.sync.dma_start(out=xk_sb[:, i], in_=xk[:, i])
        xq_sb = x_tile("xq_sb", RPC, cfg.FQ)
        for i in range(xq_sb.shape[1]):
            nc.sync.dma_start(out=xq_sb[:, i], in_=xq[:, i])
        xv_sb = []
        for kb in range(KB):
            if cfg.FV:
                t = xin.tile([128, DR, 2, 128], FP8, tag="xv", bufs=KB,
                             name=f"xv{kb}")
            else:
                t = xin.tile([128, cfg.DC, 128], BF16, tag="xv", bufs=KB,
                             name=f"xv{kb}")
            nc.sync.dma_start(out=t, in_=xv[kb])
            xv_sb.append(t)

        copy_flip = [0]

        def pcopy(out_ap, in_ap):
            # alternate psum->sbuf copies between DVE and the scalar engine
            eng = nc.vector if copy_flip[0] % 2 == 0 else nc.scalar
            copy_flip[0] += 1
            if eng is nc.vector:
                eng.tensor_copy(out=out_ap, in_=in_ap)
            else:
                eng.copy(out=out_ap, in_=in_ap)

        DRM = mybir.MatmulPerfMode.DoubleRow

        def xw_proj(x_sb, w_sb, out_sb, width, f8):
            # fp8: lhsT = w [128, 2, 128] (two stacked k-tiles), rhs = x
            # [128, 2, cols]: 256-deep contraction per DoubleRow matmul
            nch = DR if f8 else cfg.DC
            for p in range(cfg.NP):
                for c0 in range(0, width, 512):
                    cw = min(512, width - c0)
                    ps = psum.tile([128, cw], F32, tag="ctx", bufs=4,
                                   name="ps_proj")
                    for i in range(nch):
                        if f8:
                            nc.tensor.matmul(
                                ps, w_sb[:, i, :, p * 128:(p + 1) * 128],
                                x_sb[:, i, :, c0:c0 + cw],
                                start=i == 0, stop=i == nch - 1,
                                perf_mode=DRM)
                        else:
                            nc.tensor.matmul(
                                ps, w_sb[:, i, p * 128:(p + 1) * 128],
                                x_sb[:, i, c0:c0 + cw],
                                start=i == 0, stop=i == nch - 1)
                    pcopy(out_sb[:, p, c0:c0 + cw], ps)

        # K first (scores need it first), then Q.  The V projections are
        # deferred: one V unit is emitted after each of the first two
        # pairs' score/exp iterations, so the scalar engine's softmax
        # backlog overlaps the PE's V work.  V uses the "ctx" psum ring,
        # which holds no live ctx tiles yet (they allocate lazily in
        # emit_ctx, after V), so ring reuse stays acyclic.
        xw_proj(xk_sb, wk_sb, khT_sb, KPAD, cfg.FK)
        xw_proj(xq_sb, wq_sb, qhT_sb, RPC, cfg.FQ)

        nchv = DR if cfg.FV else cfg.DC

        def v_unit(kb, half):
            def run():
                ps = psum.tile([128, 512], F32, tag="ctx", bufs=4,
                               name="ps_v")
                for i in range(nchv):
                    if cfg.FV:
                        nc.tensor.matmul(
                            ps, xv_sb[kb][:, i],
                            wv_sb[:, i, :, half * 512:half * 512 + 512],
                            start=i == 0, stop=i == nchv - 1,
                            perf_mode=DRM)
                    else:
                        nc.tensor.matmul(
                            ps, xv_sb[kb][:, i],
                            wv_sb[:, i, half * 512:half * 512 + 512],
                            start=i == 0, stop=i == nchv - 1)
                pcopy(
                    vh_sb[:, kb, 8 * half:8 * half + 8, 0:dh],
                    ps.rearrange("p (h e) -> p h e", e=dh))
            return run

        v_units = [v_unit(kb, half) for kb in range(KB) for half in range(2)]

        # late transfers, in need order: pmask (first diag ~30us), then wo
        # (needed ~130us), then the LN constants — all behind the
        # critical-path weight/input streams
        pm_sb = consts.tile([128, KB, 1024], BF16)
        nc.sync.dma_start(out=pm_sb, in_=pmask)
        wo_sb = consts.tile([128, cfg.DC, D], BF16, name="wo_sb")
        for dc in range(cfg.DC):
            nc.scalar.dma_start(out=wo_sb[:, dc, :], in_=woT[:, dc, :])
        nc.sync.dma_start(out=res_sb, in_=resid)
        if gamma_bc is not None:
            nc.sync.dma_start(out=g_row, in_=gamma)
            nc.sync.dma_start(out=b_row, in_=beta)
            nc.gpsimd.partition_broadcast(gamma_bc, g_row)
            nc.gpsimd.partition_broadcast(beta_bc, b_row)

        # ---- P2: attention (pair-major, all 512 rows per tile) -------------
        ctxT_sb = proj.tile([128, cfg.NP, RPC], BF16)

        def divide(p, ctx_ps):
            # ctx rows 0..63 / den row 64; stage ctxT = ctx/den for Wo
            for h2 in range(2):
                den = small.tile([1, 512], F32, tag=f"den{h2}", bufs=2,
                                 name=f"den{h2}")
                nc.vector.tensor_copy(out=den, in_=ctx_ps[h2][dh:dh + 1, :])
                rec = small.tile([1, 512], F32, tag=f"rec{h2}", bufs=2,
                                 name=f"rec{h2}")
                nc.vector.reciprocal_approx_fast(rec, den)
                rbc = small.tile([64, 512], F32, tag=f"rbc{h2}", bufs=2,
                                 name=f"rbc{h2}")
                nc.gpsimd.partition_broadcast(rbc, rec)
                nc.vector.tensor_mul(
                    ctxT_sb[64 * h2:64 * h2 + 64, p, :],
                    ctx_ps[h2][0:dh, :], rbc)

        ctx_tiles = {}

        def emit_ctx(entry):
            kb, p, probs = entry
            if p not in ctx_tiles:
                ctx_tiles[p] = [psum.tile([dh + 1, 512], F32, tag="ctx",
                                          bufs=4, name=f"ctx{h2}")
                                for h2 in range(2)]
            ctx_ps = ctx_tiles[p]
            for h2 in range(2):
                nc.tensor.matmul(
                    ctx_ps[h2], vh_sb[:, kb, 2 * p + h2, :],
                    probs[:, h2 * 512:(h2 + 1) * 512],
                    start=kb == 0, stop=kb == KB - 1)
            if kb == KB - 1:
                divide(p, ctx_ps)
                del ctx_tiles[p]

        pend = []

        def att_iter(p, kb):
            sc = psum.tile([128, 1024], F32, tag="sc", bufs=2, name="sc")
            for h2 in range(2):
                lo = 64 * h2
                nc.tensor.matmul(
                    sc[:, h2 * 512:(h2 + 1) * 512],
                    khT_sb[lo:lo + 64, p, kb * 128:(kb + 1) * 128],
                    qhT_sb[lo:lo + 64, p, :],
                    start=True, stop=True)
            probs = att.tile([128, 1024], BF16, tag="pr", bufs=4,
                             name="probs")
            nc.scalar.activation(
                out=probs, in_=sc,
                func=mybir.ActivationFunctionType.Exp,
                scale=1.0 / math.sqrt(dh)
                / (W_SCALE if cfg.FQ else 1.0)
                / (W_SCALE if cfg.FK else 1.0))
            nc.vector.tensor_mul(probs, probs, pm_sb[:, kb, :])
            pend.append((kb, p, probs))

        for fn in v_units:
            fn()
        for p in range(cfg.NP):
            for kb in range(KB):
                att_iter(p, kb)
                if len(pend) == 3:
                    emit_ctx(pend.pop(0))
        for entry in pend:
            emit_ctx(entry)

        # ---- P3: Wo + residual + LayerNorm ---------------------------------
        for rt in range(NT):
            pso = [psum.tile([128, 512], F32, tag="ctx", bufs=4,
                             name=f"pso{ns}") for ns in range(2)]
            for p in range(cfg.NP):
                for ns in range(2):
                    nc.tensor.matmul(
                        pso[ns], ctxT_sb[:, p, rt * 128:(rt + 1) * 128],
                        wo_sb[:, p, ns * 512:ns * 512 + 512],
                        start=p == 0, stop=p == cfg.NP - 1)
            x = lnp.tile([128, D], F32, tag="x")
            for ns in range(2):
                nc.vector.tensor_add(x[:, ns * 512:ns * 512 + 512], pso[ns],
                                     res_sb[:, rt, ns * 512:ns * 512 + 512])
            fmax = math.gcd(nc.vector.BN_STATS_FMAX, D)
            nsub = D // fmax
            stats = lnp.tile([128, nsub, nc.vector.BN_STATS_DIM], F32,
                             tag="stats")
            for sg in range(nsub):
                nc.vector.bn_stats(
                    out=stats[:, sg, :],
                    in_=x.rearrange("p (a b) -> p a b", a=nsub)[:, sg, :])
            mv = lnp.tile([128, nc.vector.BN_AGGR_DIM], F32, tag="mv")
            nc.vector.bn_aggr(out=mv, in_=stats)
            sd = lnp.tile([128, 1], F32, tag="sd")
            nc.scalar.activation(out=sd, in_=mv[:, 1:2],
                                 func=mybir.ActivationFunctionType.Sqrt,
                                 bias=eps_sb, scale=1.0)
            rstd = lnp.tile([128, 1], F32, tag="rstd")
            nc.vector.reciprocal_approx_fast(rstd, sd)
            out_sb = lnp.tile([128, D], BF16, tag="out_sb")
            if cfg.G1 and cfg.B0:
                nc.vector.tensor_scalar(
                    out=out_sb, in0=x, scalar1=mv[:, 0:1], scalar2=rstd,
                    op0=mybir.AluOpType.subtract, op1=mybir.AluOpType.mult)
            else:
                y = lnp.tile([128, D], BF16, tag="y")
                nc.vector.tensor_scalar(
                    out=y, in0=x, scalar1=mv[:, 0:1], scalar2=rstd,
                    op0=mybir.AluOpType.subtract, op1=mybir.AluOpType.mult)
                if cfg.B0:
                    nc.vector.tensor_mul(out_sb, y, gamma_bc)
                elif cfg.G1:
                    nc.vector.tensor_add(out_sb, y, beta_bc)
                else:
                    yg = lnp.tile([128, D], BF16, tag="yg")
                    nc.vector.tensor_mul(yg, y, gamma_bc)
                    nc.vector.tensor_add(out_sb, yg, beta_bc)
            nc.sync.dma_start(out=out_shard[rt * 128:(rt + 1) * 128, :],
                              in_=out_sb)

    nc.compile()
    return nc


def _tile_x8(xT):
    """[D, C] f32 -> [128, D/256, 2, C] e4m3 (DoubleRow k-tile layout)."""
    d, c = xT.shape
    t = xT.reshape(d // 256, 2, 128, c).transpose(2, 0, 1, 3)
    return np.ascontiguousarray(t.astype(ml_dtypes.float8_e4m3fn))


def _tile_w(wT):
    d, o = wT.shape
    return np.ascontiguousarray(
        wT.reshape(d // 128, 128, o).transpose(1, 0, 2))


def make_in_maps(cfg: Cfg, q, k, v, Wq, Wk, Wv, Wo, gamma, beta, sen_len):
    bf = ml_dtypes.bfloat16
    q = np.asarray(q, np.float32)
    k = np.asarray(k, np.float32)
    v = np.asarray(v, np.float32)
    # fp8 weights are scaled by W_SCALE before the e4m3 cast (avoids
    # subnormals); the scale is cancelled by the exp scale (q,k) / the
    # scaled ones column (v)
    def tile_xw(xT, f8):
        return _tile_x8(xT * W_SCALE) if f8 else _tile_w(xT.astype(bf))

    wq_t = tile_xw(np.asarray(Wq, np.float32).T, cfg.FQ)
    wk_t = tile_xw(np.asarray(Wk, np.float32).T, cfg.FK)
    wv_t = tile_xw(np.asarray(Wv, np.float32).T, cfg.FV)
    wo_t = _tile_w(np.asarray(Wo, np.float32).T.astype(bf))
    g_row = np.asarray(gamma, np.float32).reshape(1, cfg.D).astype(bf)
    b_row = np.asarray(beta, np.float32).reshape(1, cfg.D).astype(bf)

    KB, KPAD, NT = cfg.KB, cfg.KPAD, cfg.NT
    per_batch = {}
    for b in range(cfg.B):
        kT_t = _tile_x8(k[b, :KPAD, :].T) if cfg.FK \
            else _tile_w(k[b, :KPAD, :].T.astype(bf))
        vT = v[b, :KPAD, :].T                     # [D, KPAD]
        if cfg.FV:
            xv = np.ascontiguousarray(
                vT.reshape(4, 2, 128, KB, 128).transpose(3, 2, 0, 1, 4)
                .astype(ml_dtypes.float8_e4m3fn))  # [KB, 128, DR, 2, 128]
        else:
            xv = np.ascontiguousarray(
                vT.astype(bf).reshape(8, 128, KB, 128)
                .transpose(2, 1, 0, 3))            # [KB, 128, DC, 128]
        per_batch[b] = (kT_t, xv)

    key_pos = np.arange(KPAD)
    in_maps = []
    for c in range(cfg.NC):
        b, qq = c // cfg.G, c % cfg.G
        tiles = [4 * j + qq for j in range(NT)]
        rows = np.concatenate(
            [np.arange(t * 128, (t + 1) * 128) for t in tiles])
        sl = int(np.asarray(sen_len)[b])

        # post-exp multiplicative mask pmask[key_p, kb, h2*512 + j*128 + f]:
        # 1 iff row (= tiles[j]*128 + f) >= key (= kb*128 + key_p) and
        # key < sen_len; identical for both h2 halves.
        rows_g = np.concatenate(
            [tiles[j] * 128 + np.arange(128) for j in range(NT)])  # [512]
        keys_g = (np.arange(KB)[:, None] * 128
                  + np.arange(128)[None, :])                       # [KB,128]
        valid = ((rows_g[None, None, :] >= keys_g[:, :, None])
                 & (keys_g[:, :, None] < sl))                      # [KB,128,512]
        pm = np.broadcast_to(
            valid.transpose(1, 0, 2)[:, :, None, :],
            (128, KB, 2, 512)).reshape(128, KB, 1024)
        pm = np.ascontiguousarray(pm.astype(ml_dtypes.bfloat16))

        xq_h = _tile_x8(q[b][rows, :].T) if cfg.FQ \
            else _tile_w(q[b][rows, :].T.astype(bf))
        res = np.ascontiguousarray(
            q[b][rows, :].reshape(NT, 128, cfg.D)
            .transpose(1, 0, 2).astype(bf))
        kT_t, xv_t = per_batch[b]
        in_maps.append({
            "xq": xq_h, "xk": kT_t, "xv": xv_t,
            "wqT": wq_t, "wkT": wk_t, "wvT": wv_t, "woT": wo_t,
            "pmask": pm, "resid": res,
            "gamma": g_row, "beta": b_row,
        })
    return in_maps


def assemble_output(cfg: Cfg, results):
    out = np.empty((cfg.B, cfg.S, cfg.D), np.float32)
    for c in range(cfg.NC):
        b, qq = c // cfg.G, c % cfg.G
        shard = results[c]["out_shard"].astype(np.float32)
        for j in range(cfg.NT):
            t = 4 * j + qq
            out[b, t * 128:(t + 1) * 128, :] = shard[j * 128:(j + 1) * 128]
    return out


_PROGRAM_CACHE = {}


def _get_program(cfg: Cfg):
    key = (cfg.B, cfg.S, cfg.D, cfg.H, cfg.dh, cfg.KB, cfg.G1, cfg.B0,
           cfg.FQ, cfg.FK, cfg.FV)
    if key not in _PROGRAM_CACHE:
        _PROGRAM_CACHE[key] = build_program(cfg)
    return _PROGRAM_CACHE[key]


def run(cfg: Cfg, inputs: dict, trace: bool = False):
    cfg.G1 = bool(np.all(np.asarray(inputs["gamma"]) == 1.0))
    cfg.B0 = bool(np.all(np.asarray(inputs["beta"]) == 0.0))
    nc = _get_program(cfg)
    in_maps = make_in_maps(cfg, **inputs)
    res = run_bass_kernel_spmd(nc, in_maps, core_ids=list(range(cfg.NC)),
                               trace=trace)
    return assemble_output(cfg, res.results), res


def kernel(**inputs) -> np.ndarray:
    kmax = int(np.max(inputs["sen_len"]))
    cfg = Cfg(B=2, S=2048, D=1024, H=16, dh=64, kmax=kmax)
    out, _ = run(cfg, inputs)
    return out


# revision 103
# speedup vs baseline: 2.0184x; 1.0457x over previous
"""Multi-head attention (QKV projections + causal/padded softmax attention +
output projection + residual + LayerNorm) as a Bass/Tile kernel on 8 Trainium2
cores — NO collectives.

Sharding: rows (sequence) are sharded across cores; every core computes ALL 16
heads for its own 512 rows end-to-end, so no cross-core communication is ever
needed.  Core c handles batch b = c//4 and the four 128-row tiles
t_j = 4*j + (c%4), j = 0..3.  The price is that each 4-core batch group
re-computes the batch's K/V projections (up to kmax keys) redundantly; that
costs ~25us of PE but saves the ~100us collective chain (CC-stream barrier +
2 AllToAlls) the head-sharded variant pays in this environment.

SPMD trick for the causal mask: the program is identical on all cores, but
the causal-diagonal position is core-dependent (rows differ per core).  ALL
masking (key padding, whole chunks above the diagonal, and the in-diagonal
triangle) is therefore one multiplicative 0/1 bf16 mask per key-chunk,
precomputed PER CORE on the host and applied to the probs AFTER the exp on
the (otherwise idle) GpSimd engine — nothing mask-related ever touches the
scores->exp critical chain, and exp of unmasked scores is safe (|s|/8 is a
few units at most).  V is augmented with a ones column so row 64 of the ctx
psum accumulates the softmax denominators for free.

Attention is pair-major: per (head-pair, key-chunk) one 512-row score matmul
per head half into separate psum banks (a hard HW constraint: matmuls whose
stationary sits at different partition bases — PE row-tile 0 vs 64 — must
not write the same PSUM bank, or the NEFF dies at runtime), one [128,1024]
exp, and two 512-row ctx matmuls accumulated over chunks.  512-wide moving
operands matter: matmul wall time is ~(moving x 0.42ns + ~145ns overhead +
LDWEIGHTS), so few big matmuls beat many small ones.

All engines are load-balanced: PE ~120us of matmul, scalar exp ~48us + half
the psum->sbuf copies, DVE the other copies + denominators + LayerNorm,
GpSimd the masks/broadcasts.  Weights stream per-dc-chunk on the scalar DMA
queue in first-use order (wk, wq, wv, then wo late); activations on the sync
queue; masks/residual/LN constants behind them.  fp8(e4m3) DoubleRow
projections were tried and measured 0.025 max-rel-err on hardware (2x the
simulator estimate) against the 0.02 gate, so everything stays bf16.
"""

import math
from contextlib import ExitStack

import numpy as np
import ml_dtypes

import concourse.mybir as mybir
import concourse.tile as tile
from concourse import bacc
from concourse.bass_utils import run_bass_kernel_spmd

BF16 = mybir.dt.bfloat16
F32 = mybir.dt.float32
FP8 = mybir.dt.float8e4
W_SCALE = 8.0  # host scales W and x->e4m3; folded back via exp scale / ones

NEG_INF = -1e9
LN_EPS = 1e-6


class Cfg:
    def __init__(self, B=2, S=2048, D=1024, H=16, dh=64, kmax=None):
        self.B, self.S, self.D, self.H, self.dh = B, S, D, H, dh
        self.kmax = S if kmax is None else max(1, min(int(kmax), S))
        self.NC = 8                       # cores
        self.G = 4                        # cores per batch group
        self.RPC = S // self.G            # rows per core (512)
        self.NT = self.RPC // 128         # row-tiles per core (4)
        self.DC = D // 128                # contraction chunks (8)
        self.NP = H // 2                  # head pairs (8)
        self.KB = -(-self.kmax // 128)    # key chunks actually needed
        self.KPAD = self.KB * 128
        # slot j covers row tile 4*j+q (q = core quarter); the static chunk
        # cap must cover the deepest core (q=3)
        self.caps = [min(4 * j + 4, self.KB) for j in range(self.NT)]
        # (j, kb) positions where ANY core's causal diagonal can fall
        self.POS = [(j, kb) for j in range(self.NT)
                    for kb in range(self.caps[j])
                    if 4 * j <= kb <= 4 * j + 3]
        # runtime-detected LN specializations
        self.G1 = False
        self.B0 = False
        # per-projection fp8 (DoubleRow) selection.  All False: e4m3
        # projections measured 0.025 max-rel-err on HW (2x the simulator's
        # 0.013) against the 0.02 gate — not worth the risk for ~15us.
        self.FQ = False
        self.FK = False
        self.FV = False


def build_program(cfg: Cfg):
    nc = bacc.Bacc("TRN2", target_bir_lowering=False, debug=False,
                   num_devices=cfg.NC)

    D, dh = cfg.D, cfg.dh
    KB, KPAD, RPC, NT = cfg.KB, cfg.KPAD, cfg.RPC, cfg.NT

    # Selected projections are pre-quantized to e4m3 on the host (weights
    # scaled by W_SCALE) and consumed by DoubleRow matmuls: 256-deep
    # contraction per instruction at 0.5 cycles/row — half the HBM bytes,
    # half the PE instructions, and half the cycles of the bf16 path.
    DR = cfg.DC // 2

    def x_in(name, cols, f8):
        if f8:
            return nc.dram_tensor(name, [128, DR, 2, cols], FP8,
                                  kind="ExternalInput").ap()
        return nc.dram_tensor(name, [128, cfg.DC, cols], BF16,
                              kind="ExternalInput").ap()

    xq = x_in("xq", RPC, cfg.FQ)
    xk = x_in("xk", KPAD, cfg.FK)
    wqT = x_in("wqT", D, cfg.FQ)
    wkT = x_in("wkT", D, cfg.FK)
    wvT = x_in("wvT", D, cfg.FV)
    if cfg.FV:
        xv = nc.dram_tensor("xv", [KB, 128, DR, 2, 128], FP8,
                            kind="ExternalInput").ap()
    else:
        xv = nc.dram_tensor("xv", [KB, 128, cfg.DC, 128], BF16,
                            kind="ExternalInput").ap()
    woT = nc.dram_tensor("woT", [128, cfg.DC, D], BF16,
                         kind="ExternalInput").ap()
    # per-core post-exp mask: pmask[key, kb, h2*512 + j*128 + f] in {0,1}
    # covers pad (key >= sen_len), causal (row < key), and above-diagonal
    # chunks, identical for every head; the 512-row block is stored twice
    # (h2 = 0/1) so one [128, 1024] multiply masks a whole pair tile.
    pmask = nc.dram_tensor("pmask", [128, KB, 1024], BF16,
                           kind="ExternalInput").ap()
    resid = nc.dram_tensor("resid", [128, NT, D], BF16,
                           kind="ExternalInput").ap()
    gamma = nc.dram_tensor("gamma", [1, D], BF16, kind="ExternalInput").ap()
    beta = nc.dram_tensor("beta", [1, D], BF16, kind="ExternalInput").ap()
    out_shard = nc.dram_tensor("out_shard", [RPC, D], BF16,
                               kind="ExternalOutput").ap()

    with tile.TileContext(nc) as tc, ExitStack() as ctx:
        consts = ctx.enter_context(tc.tile_pool(name="consts", bufs=1))
        xin = ctx.enter_context(tc.tile_pool(name="xin", bufs=1))
        proj = ctx.enter_context(tc.tile_pool(name="proj", bufs=1))
        att = ctx.enter_context(tc.tile_pool(name="att", bufs=2))
        small = ctx.enter_context(tc.tile_pool(name="small", bufs=2))
        lnp = ctx.enter_context(tc.tile_pool(name="lnp", bufs=2))
        psum = ctx.enter_context(
            tc.tile_pool(name="psum", bufs=1, space="PSUM"))

        # ---- constants -----------------------------------------------------
        # weights ride the scalar engine's DMA queue (parallel to the sync
        # queue that streams activations); per-dc chunks so the first
        # accumulation chain can start after 1/8 of the weight transfer.
        def w_tile(name, f8):
            if f8:
                return consts.tile([128, DR, 2, D], FP8, name=name)
            return consts.tile([128, cfg.DC, D], BF16, name=name)

        wk_sb = w_tile("wk_sb", cfg.FK)
        wq_sb = w_tile("wq_sb", cfg.FQ)
        wv_sb = w_tile("wv_sb", cfg.FV)
        for w_sb, w_dram in ((wk_sb, wkT), (wq_sb, wqT)):
            for i in range(w_sb.shape[1]):
                nc.scalar.dma_start(out=w_sb[:, i], in_=w_dram[:, i])
        # wv even chunks on the scalar queue (behind wk+wq); the odd chunks
        # are emitted on the sync queue after the activation stream below
        for i in range(0, wv_sb.shape[1], 2):
            nc.scalar.dma_start(out=wv_sb[:, i], in_=wvT[:, i])



        # P3 constants (loaded later on the sync queue, behind the inputs)
        gamma_bc = beta_bc = None
        if not (cfg.G1 and cfg.B0):
            g_row = consts.tile([1, D], BF16)
            b_row = consts.tile([1, D], BF16)
            gamma_bc = consts.tile([128, D], BF16)
            beta_bc = consts.tile([128, D], BF16)
        eps_sb = consts.tile([128, 1], F32)
        nc.vector.memset(eps_sb, LN_EPS)
        res_sb = consts.tile([128, NT, D], BF16)

        # ---- P1: projections ----------------------------------------------
        khT_sb = proj.tile([128, cfg.NP, KPAD], BF16)
        qhT_sb = proj.tile([128, cfg.NP, RPC], BF16)
        vh_sb = proj.tile([128, KB, cfg.H, dh + 1], BF16)
        # the ones column carries W_SCALE so ctx/den cancels the vh scale
        nc.gpsimd.memset(vh_sb[:, :, :, dh:dh + 1],
                         W_SCALE if cfg.FV else 1.0)

        # per-chunk DMAs (contiguous per partition): the first K matmul
        # only waits on chunk 0, not the whole transfer
        def x_tile(name, cols, f8):
            if f8:
                t = xin.tile([128, DR, 2, cols], FP8, name=name)
            else:
                t = xin.tile([128, cfg.DC, cols], BF16, name=name)
            return t

        xk_sb = x_tile("xk_sb", KPAD, cfg.FK)
        for i in range(xk_sb.shape[1]):
            nc.sync.dma_start(out=xk_sb[:, i], in_=xk[:, i])
        xq_sb = x_tile("xq_sb", RPC, cfg.FQ)
        for i in range(xq_sb.shape[1]):
            nc.sync.dma_start(out=xq_sb[:, i], in_=xq[:, i])
        xv_sb = []
        for kb in range(KB):
            if cfg.FV:
                t = xin.tile([128, DR, 2, 128], FP8, tag="xv", bufs=KB,
                             name=f"xv{kb}")
            else:
                t = xin.tile([128, cfg.DC, 128], BF16, tag="xv", bufs=KB,
                             name=f"xv{kb}")
            nc.sync.dma_start(out=t, in_=xv[kb])
            xv_sb.append(t)
        for i in range(1, wv_sb.shape[1], 2):
            nc.sync.dma_start(out=wv_sb[:, i], in_=wvT[:, i])

        copy_flip = [0]

        def pcopy(out_ap, in_ap):
            # alternate psum->sbuf copies between DVE and the scalar engine
            eng = nc.vector if copy_flip[0] % 2 == 0 else nc.scalar
            copy_flip[0] += 1
            if eng is nc.vector:
                eng.tensor_copy(out=out_ap, in_=in_ap)
            else:
                eng.copy(out=out_ap, in_=in_ap)

        DRM = mybir.MatmulPerfMode.DoubleRow

        def xw_proj(x_sb, w_sb, out_sb, width, f8):
            # fp8: lhsT = w [128, 2, 128] (two stacked k-tiles), rhs = x
            # [128, 2, cols]: 256-deep contraction per DoubleRow matmul
            nch = DR if f8 else cfg.DC
            for p in range(cfg.NP):
                for c0 in range(0, width, 512):
                    cw = min(512, width - c0)
                    ps = psum.tile([128, cw], F32, tag="ctx", bufs=4,
                                   name="ps_proj")
                    for i in range(nch):
                        if f8:
                            nc.tensor.matmul(
                                ps, w_sb[:, i, :, p * 128:(p + 1) * 128],
                                x_sb[:, i, :, c0:c0 + cw],
                                start=i == 0, stop=i == nch - 1,
                                perf_mode=DRM)
                        else:
                            nc.tensor.matmul(
                                ps, w_sb[:, i, p * 128:(p + 1) * 128],
                                x_sb[:, i, c0:c0 + cw],
                                start=i == 0, stop=i == nch - 1)
                    pcopy(out_sb[:, p, c0:c0 + cw], ps)

        # K first (scores need it first), then Q.  The V projections are
        # deferred: one V unit is emitted after each of the first two
        # pairs' score/exp iterations, so the scalar engine's softmax
        # backlog overlaps the PE's V work.  V uses the "ctx" psum ring,
        # which holds no live ctx tiles yet (they allocate lazily in
        # emit_ctx, after V), so ring reuse stays acyclic.
        xw_proj(xk_sb, wk_sb, khT_sb, KPAD, cfg.FK)
        xw_proj(xq_sb, wq_sb, qhT_sb, RPC, cfg.FQ)

        nchv = DR if cfg.FV else cfg.DC

        def v_unit(kb, half):
            def run():
                ps = psum.tile([128, 512], F32, tag="ctx", bufs=4,
                               name="ps_v")
                for i in range(nchv):
                    if cfg.FV:
                        nc.tensor.matmul(
                            ps, xv_sb[kb][:, i],
                            wv_sb[:, i, :, half * 512:half * 512 + 512],
                            start=i == 0, stop=i == nchv - 1,
                            perf_mode=DRM)
                    else:
                        nc.tensor.matmul(
                            ps, xv_sb[kb][:, i],
                            wv_sb[:, i, half * 512:half * 512 + 512],
                            start=i == 0, stop=i == nchv - 1)
                pcopy(
                    vh_sb[:, kb, 8 * half:8 * half + 8, 0:dh],
                    ps.rearrange("p (h e) -> p h e", e=dh))
            return run

        v_units = [v_unit(kb, half) for kb in range(KB) for half in range(2)]

        # late transfers, in need order: pmask (first diag ~30us), then wo
        # (needed ~130us), then the LN constants — all behind the
        # critical-path weight/input streams
        pm_sb = consts.tile([128, KB, 1024], BF16)
        nc.sync.dma_start(out=pm_sb, in_=pmask)
        wo_sb = consts.tile([128, cfg.DC, D], BF16, name="wo_sb")
        for dc in range(cfg.DC):
            nc.scalar.dma_start(out=wo_sb[:, dc, :], in_=woT[:, dc, :])
        nc.sync.dma_start(out=res_sb, in_=resid)
        if gamma_bc is not None:
            nc.sync.dma_start(out=g_row, in_=gamma)
            nc.sync.dma_start(out=b_row, in_=beta)
            nc.gpsimd.partition_broadcast(gamma_bc, g_row)
            nc.gpsimd.partition_broadcast(beta_bc, b_row)

        # ---- P2: attention (pair-major, all 512 rows per tile) -------------
        ctxT_sb = proj.tile([128, cfg.NP, RPC], BF16)

        def divide(p, ctx_ps):
            # ctx rows 0..63 / den row 64; stage ctxT = ctx/den for Wo
            for h2 in range(2):
                den = small.tile([1, 512], F32, tag=f"den{h2}", bufs=2,
                                 name=f"den{h2}")
                nc.vector.tensor_copy(out=den, in_=ctx_ps[h2][dh:dh + 1, :])
                rec = small.tile([1, 512], F32, tag=f"rec{h2}", bufs=2,
                                 name=f"rec{h2}")
                nc.vector.reciprocal_approx_fast(rec, den)
                rbc = small.tile([64, 512], F32, tag=f"rbc{h2}", bufs=2,
                                 name=f"rbc{h2}")
                nc.gpsimd.partition_broadcast(rbc, rec)
                nc.vector.tensor_mul(
                    ctxT_sb[64 * h2:64 * h2 + 64, p, :],
                    ctx_ps[h2][0:dh, :], rbc)

        ctx_tiles = {}

        def emit_ctx(entry):
            kb, p, probs = entry
            if p not in ctx_tiles:
                ctx_tiles[p] = [psum.tile([dh + 1, 512], F32, tag="ctx",
                                          bufs=4, name=f"ctx{h2}")
                                for h2 in range(2)]
            ctx_ps = ctx_tiles[p]
            for h2 in range(2):
                nc.tensor.matmul(
                    ctx_ps[h2], vh_sb[:, kb, 2 * p + h2, :],
                    probs[:, h2 * 512:(h2 + 1) * 512],
                    start=kb == 0, stop=kb == KB - 1)
            if kb == KB - 1:
                divide(p, ctx_ps)
                del ctx_tiles[p]

        pend = []

        def att_iter(p, kb):
            sc = psum.tile([128, 1024], F32, tag="sc", bufs=2, name="sc")
            for h2 in range(2):
                lo = 64 * h2
                nc.tensor.matmul(
                    sc[:, h2 * 512:(h2 + 1) * 512],
                    khT_sb[lo:lo + 64, p, kb * 128:(kb + 1) * 128],
                    qhT_sb[lo:lo + 64, p, :],
                    start=True, stop=True)
            probs = att.tile([128, 1024], BF16, tag="pr", bufs=6,
                             name="probs")
            nc.scalar.activation(
                out=probs, in_=sc,
                func=mybir.ActivationFunctionType.Exp,
                scale=1.0 / math.sqrt(dh)
                / (W_SCALE if cfg.FQ else 1.0)
                / (W_SCALE if cfg.FK else 1.0))
            nc.vector.tensor_mul(probs, probs, pm_sb[:, kb, :])
            pend.append((kb, p, probs))

        for fn in v_units:
            fn()
        for p in range(cfg.NP):
            for kb in range(KB):
                att_iter(p, kb)
                if len(pend) == 5:
                    emit_ctx(pend.pop(0))
        for entry in pend:
            emit_ctx(entry)

        # ---- P3: Wo + residual + LayerNorm ---------------------------------
        for rt in range(NT):
            pso = [psum.tile([128, 512], F32, tag="ctx", bufs=4,
                             name=f"pso{ns}") for ns in range(2)]
            for p in range(cfg.NP):
                for ns in range(2):
                    nc.tensor.matmul(
                        pso[ns], ctxT_sb[:, p, rt * 128:(rt + 1) * 128],
                        wo_sb[:, p, ns * 512:ns * 512 + 512],
                        start=p == 0, stop=p == cfg.NP - 1)
            x = lnp.tile([128, D], F32, tag="x")
            for ns in range(2):
                nc.vector.tensor_add(x[:, ns * 512:ns * 512 + 512], pso[ns],
                                     res_sb[:, rt, ns * 512:ns * 512 + 512])
            fmax = math.gcd(nc.vector.BN_STATS_FMAX, D)
            nsub = D // fmax
            stats = lnp.tile([128, nsub, nc.vector.BN_STATS_DIM], F32,
                             tag="stats")
            for sg in range(nsub):
                nc.vector.bn_stats(
                    out=stats[:, sg, :],
                    in_=x.rearrange("p (a b) -> p a b", a=nsub)[:, sg, :])
            mv = lnp.tile([128, nc.vector.BN_AGGR_DIM], F32, tag="mv")
            nc.vector.bn_aggr(out=mv, in_=stats)
            sd = lnp.tile([128, 1], F32, tag="sd")
            nc.scalar.activation(out=sd, in_=mv[:, 1:2],
                                 func=mybir.ActivationFunctionType.Sqrt,
                                 bias=eps_sb, scale=1.0)
            rstd = lnp.tile([128, 1], F32, tag="rstd")
            nc.vector.reciprocal_approx_fast(rstd, sd)
            out_sb = lnp.tile([128, D], BF16, tag="out_sb")
            if cfg.G1 and cfg.B0:
                nc.vector.tensor_scalar(
                    out=out_sb, in0=x, scalar1=mv[:, 0:1], scalar2=rstd,
                    op0=mybir.AluOpType.subtract, op1=mybir.AluOpType.mult)
            else:
                y = lnp.tile([128, D], BF16, tag="y")
                nc.vector.tensor_scalar(
                    out=y, in0=x, scalar1=mv[:, 0:1], scalar2=rstd,
                    op0=mybir.AluOpType.subtract, op1=mybir.AluOpType.mult)
                if cfg.B0:
                    nc.vector.tensor_mul(out_sb, y, gamma_bc)
                elif cfg.G1:
                    nc.vector.tensor_add(out_sb, y, beta_bc)
                else:
                    yg = lnp.tile([128, D], BF16, tag="yg")
                    nc.vector.tensor_mul(yg, y, gamma_bc)
                    nc.vector.tensor_add(out_sb, yg, beta_bc)
            nc.sync.dma_start(out=out_shard[rt * 128:(rt + 1) * 128, :],
                              in_=out_sb)

    nc.compile()
    return nc


def _tile_x8(xT):
    """[D, C] f32 -> [128, D/256, 2, C] e4m3 (DoubleRow k-tile layout)."""
    d, c = xT.shape
    t = xT.reshape(d // 256, 2, 128, c).transpose(2, 0, 1, 3)
    return np.ascontiguousarray(t.astype(ml_dtypes.float8_e4m3fn))


def _tile_w(wT):
    d, o = wT.shape
    return np.ascontiguousarray(
        wT.reshape(d // 128, 128, o).transpose(1, 0, 2))


def make_in_maps(cfg: Cfg, q, k, v, Wq, Wk, Wv, Wo, gamma, beta, sen_len):
    bf = ml_dtypes.bfloat16
    q = np.asarray(q, np.float32)
    k = np.asarray(k, np.float32)
    v = np.asarray(v, np.float32)
    # fp8 weights are scaled by W_SCALE before the e4m3 cast (avoids
    # subnormals); the scale is cancelled by the exp scale (q,k) / the
    # scaled ones column (v)
    def tile_xw(xT, f8):
        return _tile_x8(xT * W_SCALE) if f8 else _tile_w(xT.astype(bf))

    wq_t = tile_xw(np.asarray(Wq, np.float32).T, cfg.FQ)
    wk_t = tile_xw(np.asarray(Wk, np.float32).T, cfg.FK)
    wv_t = tile_xw(np.asarray(Wv, np.float32).T, cfg.FV)
    wo_t = _tile_w(np.asarray(Wo, np.float32).T.astype(bf))
    g_row = np.asarray(gamma, np.float32).reshape(1, cfg.D).astype(bf)
    b_row = np.asarray(beta, np.float32).reshape(1, cfg.D).astype(bf)

    KB, KPAD, NT = cfg.KB, cfg.KPAD, cfg.NT
    per_batch = {}
    for b in range(cfg.B):
        kT_t = _tile_x8(k[b, :KPAD, :].T) if cfg.FK \
            else _tile_w(k[b, :KPAD, :].T.astype(bf))
        vT = v[b, :KPAD, :].T                     # [D, KPAD]
        if cfg.FV:
            xv = np.ascontiguousarray(
                vT.reshape(4, 2, 128, KB, 128).transpose(3, 2, 0, 1, 4)
                .astype(ml_dtypes.float8_e4m3fn))  # [KB, 128, DR, 2, 128]
        else:
            xv = np.ascontiguousarray(
                vT.astype(bf).reshape(8, 128, KB, 128)
                .transpose(2, 1, 0, 3))            # [KB, 128, DC, 128]
        per_batch[b] = (kT_t, xv)

    key_pos = np.arange(KPAD)
    in_maps = []
    for c in range(cfg.NC):
        b, qq = c // cfg.G, c % cfg.G
        tiles = [4 * j + qq for j in range(NT)]
        rows = np.concatenate(
            [np.arange(t * 128, (t + 1) * 128) for t in tiles])
        sl = int(np.asarray(sen_len)[b])

        # post-exp multiplicative mask pmask[key_p, kb, h2*512 + j*128 + f]:
        # 1 iff row (= tiles[j]*128 + f) >= key (= kb*128 + key_p) and
        # key < sen_len; identical for both h2 halves.
        rows_g = np.concatenate(
            [tiles[j] * 128 + np.arange(128) for j in range(NT)])  # [512]
        keys_g = (np.arange(KB)[:, None] * 128
                  + np.arange(128)[None, :])                       # [KB,128]
        valid = ((rows_g[None, None, :] >= keys_g[:, :, None])
                 & (keys_g[:, :, None] < sl))                      # [KB,128,512]
        pm = np.broadcast_to(
            valid.transpose(1, 0, 2)[:, :, None, :],
            (128, KB, 2, 512)).reshape(128, KB, 1024)
        pm = np.ascontiguousarray(pm.astype(ml_dtypes.bfloat16))

        xq_h = _tile_x8(q[b][rows, :].T) if cfg.FQ \
            else _tile_w(q[b][rows, :].T.astype(bf))
        res = np.ascontiguousarray(
            q[b][rows, :].reshape(NT, 128, cfg.D)
            .transpose(1, 0, 2).astype(bf))
        kT_t, xv_t = per_batch[b]
        in_maps.append({
            "xq": xq_h, "xk": kT_t, "xv": xv_t,
            "wqT": wq_t, "wkT": wk_t, "wvT": wv_t, "woT": wo_t,
            "pmask": pm, "resid": res,
            "gamma": g_row, "beta": b_row,
        })
    return in_maps


def assemble_output(cfg: Cfg, results):
    out = np.empty((cfg.B, cfg.S, cfg.D), np.float32)
    for c in range(cfg.NC):
        b, qq = c // cfg.G, c % cfg.G
        shard = results[c]["out_shard"].astype(np.float32)
        for j in range(cfg.NT):
            t = 4 * j + qq
            out[b, t * 128:(t + 1) * 128, :] = shard[j * 128:(j + 1) * 128]
    return out


_PROGRAM_CACHE = {}


def _get_program(cfg: Cfg):
    key = (cfg.B, cfg.S, cfg.D, cfg.H, cfg.dh, cfg.KB, cfg.G1, cfg.B0,
           cfg.FQ, cfg.FK, cfg.FV)
    if key not in _PROGRAM_CACHE:
        _PROGRAM_CACHE[key] = build_program(cfg)
    return _PROGRAM_CACHE[key]


def run(cfg: Cfg, inputs: dict, trace: bool = False):
    cfg.G1 = bool(np.all(np.asarray(inputs["gamma"]) == 1.0))
    cfg.B0 = bool(np.all(np.asarray(inputs["beta"]) == 0.0))
    nc = _get_program(cfg)
    in_maps = make_in_maps(cfg, **inputs)
    res = run_bass_kernel_spmd(nc, in_maps, core_ids=list(range(cfg.NC)),
                               trace=trace)
    return assemble_output(cfg, res.results), res


def kernel(**inputs) -> np.ndarray:
    kmax = int(np.max(inputs["sen_len"]))
    cfg = Cfg(B=2, S=2048, D=1024, H=16, dh=64, kmax=kmax)
    out, _ = run(cfg, inputs)
    return out
